# revision 18
# baseline (speedup 1.0000x reference)
"""FAGCN forward on 8 TRN2 NeuronCores (Bass/Tile).

Sharding: row-partition of nodes, 8 ways. Each core projects only its own
rows (h = relu(x @ t1^T + b)) into 512B gather records
[dinv*h bf16 x128 | b bf16 replicated x128]; an AllGather assembles the
full table (same path builds the layer-1 table from finalized windows).
Per layer the edge phase is a two-stream token walk ([all-lo windows]
[all-hi windows], int16 gather indices split at lo_split): 1024-edge
SWDGE dma_gather calls fetch source records. The edge weight never
materializes per-token: the gate runs dest-expanded,
ohc[p,c,f] = (iota[f]==rr[p,c]) * tanh(awb[p,f] + brep[p,c,f]) * dinvw[f],
where every DVE operand is stride-1 in the innermost dim (2x mode), tanh
runs on the scalar engine, awb is the per-window a-vector broadcast built
by two small matmuls, and padding tokens carry rr=-1 so the equality mask
kills them. The scatter-add is a TensorE matmul per 128-token tile into a
per-window PSUM accumulator. The head (t2 matmul + log_softmax) is a
final two-pass sweep.
"""

import os
import sys
import numpy as np

sys.path.insert(0, "/opt/trn_rl_repo")

import concourse.bass as bass
import concourse.bacc as bacc
import concourse.mybir as mybir
import concourse.tile as tile
from concourse import library_config

F32 = mybir.dt.float32
BF16 = mybir.dt.bfloat16
I16 = mybir.dt.int16

# problem constants (self-contained per contract)
N_NODES = 50000
IN_CH = 256
HIDDEN = 128
OUT_CH = 64
EPS = 0.3
NCORES = 8
CALL_TOKENS = int(os.environ.get("KCT", "1024"))
CT_MAX = CALL_TOKENS // 128
EXT_SLOTS = 128   # 512B gather record
B_SLOT = 64       # first f32 slot of the bf16-replicated b region
PREP_GRP = 8


def _install_profile_hook():
    import types
    name = "antenv.axon_hooks"
    if name in sys.modules:
        return
    try:
        import trn_agent_boot.trn_boot as tb
        hook = tb._ntff_profile_via_ctypes("/opt/axon/libaxon_pjrt.so")
    except Exception:
        hook = None
    mod = types.ModuleType(name)
    mod._hook = hook
    mod.get_axon_ntff_profile_hook = lambda: mod._hook
    mod.set_axon_ntff_profile_hook = lambda h: setattr(mod, "_hook", h)
    sys.modules[name] = mod


# ======================================================================
# Host preprocessing: SPMD token streams + per-core data
# ======================================================================

def preprocess(edge_index, n_nodes, ncores, lo_split):
    row = np.asarray(edge_index[0], dtype=np.int64)
    col = np.asarray(edge_index[1], dtype=np.int64)
    E = row.shape[0]
    r_per = n_nodes // ncores
    nwin = (r_per + 127) // 128

    deg = np.bincount(row, minlength=n_nodes).astype(np.float64)
    dinv = np.where(deg > 0, 1.0 / np.sqrt(np.maximum(deg, 1.0)), 0.0).astype(np.float32)

    core = row // r_per
    lrow = row - core * r_per
    win = lrow // 128
    is_hi = (col >= lo_split).astype(np.int64)

    # stream order: core, then stream (lo/hi), then window, then lrow
    order = np.lexsort((lrow, win, is_hi, core))
    core_s, win_s, hi_s = core[order], win[order], is_hi[order]
    lrow_s, col_s = lrow[order], col[order]

    key = (core_s * 2 + hi_s) * nwin + win_s
    cnt = np.bincount(key, minlength=ncores * 2 * nwin).reshape(ncores, 2, nwin)
    sec_len = ((cnt.max(axis=0) + 127) // 128) * 128  # [2, nwin]
    L_lo = int(sec_len[0].sum())
    L_hi = int(sec_len[1].sum())
    e_tok = L_lo + L_hi
    sec_start = np.zeros((2, nwin), np.int64)
    sec_start[0] = np.concatenate([[0], np.cumsum(sec_len[0])[:-1]])
    sec_start[1] = L_lo + np.concatenate([[0], np.cumsum(sec_len[1])[:-1]])

    col16 = np.zeros((ncores, e_tok), np.int16)
    rowrel = np.full((ncores, e_tok), -1.0, np.float32)  # -1 = padding (mask)

    grp_first = np.zeros(ncores * 2 * nwin + 1, np.int64)
    np.cumsum(cnt.reshape(-1), out=grp_first[1:])
    rank = np.arange(E) - grp_first[key]
    dest = sec_start[hi_s, win_s] + rank
    cval = np.where(hi_s == 1, col_s - lo_split, col_s).astype(np.int16)
    col16[core_s, dest] = cval
    rowrel[core_s, dest] = (lrow_s - win_s * 128).astype(np.float32)

    # gather calls per stream
    calls = []  # (stream, ts, nt)
    for h, base, L in ((0, 0, L_lo), (1, L_lo, L_hi)):
        off = 0
        while off < L:
            nt = min(CALL_TOKENS, L - off)
            calls.append((h, base + off, nt))
            off += nt

    idx_dev = np.zeros((ncores, 128, e_tok // 16), np.int16)
    for (h, ts, nt) in calls:
        blk = col16[:, ts:ts + nt].reshape(ncores, nt // 16, 16)
        blk = np.ascontiguousarray(np.transpose(blk, (0, 2, 1)))
        idx_dev[:, :, ts // 16:(ts + nt) // 16] = np.tile(blk, (1, 8, 1))
    rr_dev = np.ascontiguousarray(rowrel.reshape(ncores, -1, 128).transpose(0, 2, 1))

    return {
        "nwin": nwin, "e_tok": e_tok, "sec_len": sec_len, "calls": calls,
        "idx_dev": idx_dev, "rr_dev": rr_dev, "dinv": dinv,
    }


# ======================================================================
# Kernel builder
# ======================================================================

def build_kernel(meta, n_nodes, in_ch, hidden, out_ch, eps, lo_split, ncores):
    nwin = meta["nwin"]
    e_tok = meta["e_tok"]
    sec_len = meta["sec_len"]
    calls = meta["calls"]
    r_per = n_nodes // ncores
    last_win_rows = r_per - 128 * (nwin - 1)
    kt = in_ch // 128
    hh = hidden // 2  # f32 slots holding the bf16 h vector

    # tile -> window map, and burst boundaries per (stream, window)
    tiles_w = []
    burst = {}  # (h, w) -> (gfirst, glast) in global tile idx
    for h in range(2):
        for w in range(nwin):
            ntl = int(sec_len[h, w]) // 128
            if ntl == 0:
                continue
            g0 = len(tiles_w)
            tiles_w.extend([w] * ntl)
            burst[(h, w)] = (g0, g0 + ntl - 1)
    assert len(tiles_w) == e_tok // 128
    last_stream = {}
    for w in range(nwin):
        last_stream[w] = 1 if (1, w) in burst else 0

    ogrp = nwin // PREP_GRP
    ogrp_rem = nwin - ogrp * PREP_GRP

    nc = bacc.Bacc("TRN2", target_bir_lowering=False, debug=False,
                   num_devices=ncores, num_swdge_queues=4)

    # ---- I/O ----
    xtog = nc.dram_tensor("xtog", [ogrp + (1 if ogrp_rem else 0), 128, PREP_GRP * kt, 128], BF16, kind="ExternalInput")
    xbog = nc.dram_tensor("xbog", [ogrp + (1 if ogrp_rem else 0), 1, PREP_GRP, 128], BF16, kind="ExternalInput")
    t1wt = nc.dram_tensor("t1wt", [in_ch + 1, hidden], BF16, kind="ExternalInput")
    gwrep = nc.dram_tensor("gwrep", [4, 128, hidden], BF16, kind="ExternalInput")
    gbrep = nc.dram_tensor("gbrep", [128, 2], F32, kind="ExternalInput")
    t2wt = nc.dram_tensor("t2wt", [hidden, out_ch], F32, kind="ExternalInput")
    t2b = nc.dram_tensor("t2b", [1, out_ch], F32, kind="ExternalInput")
    iotac_in = nc.dram_tensor("iotac", [128, CT_MAX * 128], BF16, kind="ExternalInput")
    ident_in = nc.dram_tensor("ident", [128, 128], F32, kind="ExternalInput")
    identb_in = nc.dram_tensor("identb", [128, 128], BF16, kind="ExternalInput")
    ones_in = nc.dram_tensor("ones", [1, 128], F32, kind="ExternalInput")
    ones128b_in = nc.dram_tensor("ones128b", [128, 128], BF16, kind="ExternalInput")
    idx_in = nc.dram_tensor("idx", [128, e_tok // 16], I16, kind="ExternalInput")
    rrb_in = nc.dram_tensor("rrb", [128, e_tok // 128], BF16, kind="ExternalInput")
    dinvw_in = nc.dram_tensor("dinvw", [128, nwin * 128], BF16, kind="ExternalInput")
    dinvc_in = nc.dram_tensor("dinvc", [128, nwin], F32, kind="ExternalInput")
    out = nc.dram_tensor("out", [r_per, out_ch], F32, kind="ExternalOutput")

    ext0 = nc.dram_tensor("ext0", [r_per * ncores, EXT_SLOTS], F32)
    agi0 = nc.dram_tensor("agi0", [r_per, EXT_SLOTS], F32)
    agi = nc.dram_tensor("agi", [r_per, EXT_SLOTS], F32)
    ago = nc.dram_tensor("ago", [r_per * ncores, EXT_SLOTS], F32)

    with tile.TileContext(nc) as tc:
        nc.gpsimd.load_library(library_config.mlp)
        with tc.tile_pool(name="consts", bufs=1) as cp:
            t1wt_sb = cp.tile([128, kt, hidden], BF16, tag="t1wt")
            nc.sync.dma_start(t1wt_sb[:], bass.AP(t1wt, 0, [[hidden, 128], [128 * hidden, kt], [1, hidden]]))
            t1b_sb = cp.tile([1, hidden], BF16, tag="t1b")
            nc.sync.dma_start(t1b_sb[:], t1wt.ap()[in_ch:in_ch + 1, :])
            gw_sb = cp.tile([128, 4, hidden], BF16, tag="gw")
            nc.sync.dma_start(gw_sb[:], bass.AP(gwrep, 0, [[hidden, 128], [128 * hidden, 4], [1, hidden]]))
            gb_sb = cp.tile([128, 2], F32, tag="gb")
            nc.sync.dma_start(gb_sb[:], gbrep.ap())
            t2wt_sb = cp.tile([128, out_ch], F32, tag="t2wt")
            nc.sync.dma_start(t2wt_sb[:], t2wt.ap())
            t2b_sb = cp.tile([1, out_ch], F32, tag="t2b")
            nc.sync.dma_start(t2b_sb[:], t2b.ap())
            ident_sb = cp.tile([128, 128], F32, tag="ident")
            nc.sync.dma_start(ident_sb[:], ident_in.ap())
            identb_sb = cp.tile([128, 128], BF16, tag="identb")
            nc.sync.dma_start(identb_sb[:], identb_in.ap())
            ones_sb = cp.tile([1, 128], F32, tag="ones")
            nc.sync.dma_start(ones_sb[:], ones_in.ap())
            ones128b_sb = cp.tile([128, 128], BF16, tag="ones128b")
            nc.sync.dma_start(ones128b_sb[:], ones128b_in.ap())
            idxt = cp.tile([128, e_tok // 16], I16, tag="idxt")
            nc.sync.dma_start(idxt[:], idx_in.ap())
            rrb_sb = cp.tile([128, e_tok // 128], BF16, tag="rrb")
            nc.sync.dma_start(rrb_sb[:], rrb_in.ap())
            dinvw_sb = cp.tile([128, nwin, 128], BF16, tag="dinvw")
            nc.sync.dma_start(dinvw_sb[:], dinvw_in.ap())
            dinvc_sb = cp.tile([128, nwin], F32, tag="dinvc")
            nc.sync.dma_start(dinvc_sb[:], dinvc_in.ap())
            iotac_sb = cp.tile([128, CT_MAX, 128], BF16, tag="iotac")
            nc.sync.dma_start(iotac_sb[:], iotac_in.ap())

            rawsc = cp.tile([128, nwin, hidden], F32, tag="rawsc")
            acc = cp.tile([128, nwin, hidden], F32, tag="acc")
            a_arr = cp.tile([128, nwin, 2], BF16, tag="a_arr")

            # ------- prep: own rows only -> records -> agi0 (AllGather later)
            with tc.tile_pool(name="prep", bufs=3) as pp, \
                 tc.tile_pool(name="prep_s", bufs=6) as pscr, \
                 tc.tile_pool(name="prep_ps", bufs=4, space="PSUM") as pps:

                def prep_group(gi, gcnt):
                    xt_sb = pp.tile([128, PREP_GRP * kt, 128], BF16, tag="xt")
                    nc.sync.dma_start(xt_sb[:, 0:gcnt * kt, :], xtog.ap()[gi, :, 0:gcnt * kt, :])
                    xb_sb = pp.tile([1, PREP_GRP, 128], BF16, tag="xb")
                    nc.sync.dma_start(xb_sb[:, 0:gcnt, :], xbog.ap()[gi, :, 0:gcnt, :])
                    for c in range(gcnt):
                        w = gi * PREP_GRP + c
                        rows = 128 if w < nwin - 1 else last_win_rows
                        ps = pps.tile([128, hidden], F32, tag="h0ps")
                        for k in range(kt):
                            nc.tensor.matmul(ps[:], xt_sb[:, c * kt + k, :], t1wt_sb[:, k, :],
                                             start=(k == 0), stop=False)
                        nc.tensor.matmul(ps[:], xb_sb[:, c, :], t1b_sb[:], start=False, stop=True)
                        nc.vector.tensor_scalar(out=rawsc[:, w, :], in0=ps[:],
                                                scalar1=0.0, scalar2=eps,
                                                op0=mybir.AluOpType.max,
                                                op1=mybir.AluOpType.mult)
                        extg = pp.tile([128, EXT_SLOTS], F32, tag="extg")
                        # record payload: dinv * relu(h)  (dinv >= 0 commutes with relu)
                        nc.scalar.activation(extg[:, 0:hh].bitcast(BF16), ps[:],
                                             mybir.ActivationFunctionType.Relu,
                                             scale=dinvc_sb[:, w:w + 1])
                        # gate terms from unscaled relu(h): STT does the relu via max
                        scr = pscr.tile([128, hidden], BF16, tag="scr")
                        a_f = pscr.tile([128, 1], F32, tag="af")
                        nc.vector.scalar_tensor_tensor(
                            out=scr[:], in0=ps[:], scalar=0.0, in1=gw_sb[:, 0, :],
                            op0=mybir.AluOpType.max, op1=mybir.AluOpType.mult,
                            accum_out=a_f[:])
                        nc.vector.tensor_scalar(out=a_arr[:, w, 0:1], in0=a_f[:],
                                                scalar1=gb_sb[:, 0:1], scalar2=None,
                                                op0=mybir.AluOpType.add)
                        scr2 = pscr.tile([128, hidden], BF16, tag="scr2")
                        b_f = pscr.tile([128, 1], F32, tag="bf")
                        nc.vector.scalar_tensor_tensor(
                            out=scr2[:], in0=ps[:], scalar=0.0, in1=gw_sb[:, 1, :],
                            op0=mybir.AluOpType.max, op1=mybir.AluOpType.mult,
                            accum_out=b_f[:])
                        nc.scalar.activation(extg[:, B_SLOT:].bitcast(BF16), ones128b_sb[:],
                                             mybir.ActivationFunctionType.Copy,
                                             scale=b_f[:])
                        nc.sync.dma_start(agi0.ap()[w * 128:w * 128 + rows, :],
                                          extg[0:rows, :])

                for gi in range(ogrp + (1 if ogrp_rem else 0)):
                    prep_group(gi, PREP_GRP if gi < ogrp else ogrp_rem)

            # ---------------- edge phase (per layer) ----------------
            def emit_layer(l, table):
                lo_ap = table.ap()
                hi_ap = table.ap()[lo_split:, :]
                awb = cp.tile([128, nwin, 128], BF16, tag=f"awb{l}")
                with tc.tile_pool(name=f"g{l}", bufs=int(os.environ.get("KGB", "4"))) as gp, \
                     tc.tile_pool(name=f"scr{l}", bufs=6) as scrp, \
                     tc.tile_pool(name=f"oh{l}", bufs=3) as ohp, \
                     tc.tile_pool(name=f"fin{l}", bufs=2) as fp, \
                     tc.tile_pool(name=f"aw{l}", bufs=4) as awp, \
                     tc.tile_pool(name=f"awps{l}", bufs=2, space="PSUM") as awps, \
                     tc.tile_pool(name=f"psW{l}", bufs=2, space="PSUM") as psW:
                    # per-window broadcast of the a-term: awb[p, w, f] = a_arr[f, w, l]
                    for w in range(nwin):
                        tp_ps = awps.tile([1, 128], F32, tag="aT")
                        nc.tensor.matmul(tp_ps[:], a_arr[:, w, l:l + 1], identb_sb[:],
                                         start=True, stop=True)
                        aT = awp.tile([1, 128], F32, tag="aTs")
                        nc.vector.tensor_copy(aT[:], tp_ps[:])
                        ab_ps = awps.tile([128, 128], F32, tag="ab")
                        nc.tensor.matmul(ab_ps[:], ones_sb[:], aT[:],
                                         start=True, stop=True)
                        nc.scalar.activation(awb[:, w, :], ab_ps[:],
                                             mybir.ActivationFunctionType.Copy)

                    def finalize(w):
                        rows = 128 if w < nwin - 1 else last_win_rows
                        if l == 0:
                            ext1 = fp.tile([128, EXT_SLOTS], F32, tag="ext1")
                            nc.scalar.activation(ext1[:, 0:hh].bitcast(BF16), acc[:, w, :],
                                                 mybir.ActivationFunctionType.Copy,
                                                 scale=dinvc_sb[:, w:w + 1])
                            scr = scrp.tile([128, hidden], BF16, tag="escr")
                            b_f = scrp.tile([128, 1], F32, tag="bf1")
                            nc.vector.scalar_tensor_tensor(
                                out=scr[:], in0=acc[:, w, :], scalar=1.0, in1=gw_sb[:, 3, :],
                                op0=mybir.AluOpType.mult, op1=mybir.AluOpType.mult,
                                accum_out=b_f[:])
                            nc.scalar.activation(ext1[:, B_SLOT:].bitcast(BF16),
                                                 ones128b_sb[:],
                                                 mybir.ActivationFunctionType.Copy,
                                                 scale=b_f[:])
                            scr2 = scrp.tile([128, hidden], BF16, tag="escr2")
                            a_f = scrp.tile([128, 1], F32, tag="af1")
                            nc.vector.scalar_tensor_tensor(
                                out=scr2[:], in0=acc[:, w, :], scalar=1.0, in1=gw_sb[:, 2, :],
                                op0=mybir.AluOpType.mult, op1=mybir.AluOpType.mult,
                                accum_out=a_f[:])
                            nc.vector.tensor_scalar(out=a_arr[:, w, 1:2], in0=a_f[:],
                                                    scalar1=gb_sb[:, 1:2], scalar2=None,
                                                    op0=mybir.AluOpType.add)
                            nc.sync.dma_start(agi.ap()[w * 128:w * 128 + rows, :],
                                              ext1[0:rows, :])

                    # windows with no lo-burst: seed acc with rawsc; fully
                    # edgeless windows also finalize immediately
                    for w in range(nwin):
                        if (0, w) not in burst:
                            nc.vector.tensor_copy(acc[:, w, :], rawsc[:, w, :])
                            if (1, w) not in burst:
                                finalize(w)

                    qi = 0
                    W_ps = None
                    for (h, ts, nt) in [c for c in calls]:
                        ct = nt // 128
                        t0 = ts // 128
                        G = gp.tile([128, CT_MAX, EXT_SLOTS], F32, tag="G")
                        nc.gpsimd.dma_gather(
                            out_ap=G[:, 0:ct, :],
                            in_ap=(hi_ap if h else lo_ap),
                            idxs_ap=idxt[:, ts // 16:(ts + nt) // 16],
                            num_idxs=nt, num_idxs_reg=nt, elem_size=EXT_SLOTS,
                            single_packet=False, queue_num=qi % 4)
                        qi += 1
                        # split call into window-pure runs
                        runs = []
                        c = 0
                        while c < ct:
                            w = tiles_w[t0 + c]
                            c1 = c
                            while c1 < ct and tiles_w[t0 + c1] == w:
                                c1 += 1
                            runs.append((w, c, c1))
                            c = c1
                        # mask: eqc[p, c, f] = (iota[f]==rr[p,c]); rr=-1 pads
                        eqc = ohp.tile([128, CT_MAX, 128], BF16, tag="eqc")
                        rrs = rrb_sb[:, t0:t0 + ct]
                        rrx = bass.AP(rrs.tensor, rrs.offset, list(rrs.ap) + [[0, 128]])
                        nc.vector.tensor_tensor(out=eqc[:, 0:ct, :],
                                                in0=iotac_sb[:, 0:ct, :], in1=rrx,
                                                op=mybir.AluOpType.is_equal)
                        # dest-expanded gate arg: s1 = awb[p,f] + b[p,c]
                        s1 = ohp.tile([128, CT_MAX, 128], BF16, tag="s1")
                        for (w, c0, c1) in runs:
                            rK = c1 - c0
                            aws = awb[:, w, :]
                            awx = bass.AP(aws.tensor, aws.offset,
                                          [list(aws.ap)[0], [0, rK], list(aws.ap)[1]])
                            bx = G[:, c0:c1, B_SLOT:].bitcast(BF16)
                            nc.vector.tensor_tensor(out=s1[:, c0:c1, :],
                                                    in0=bx, in1=awx,
                                                    op=mybir.AluOpType.add)
                        th = ohp.tile([128, CT_MAX, 128], BF16, tag="th")
                        nc.scalar.activation(th[:, 0:ct, :], s1[:, 0:ct, :],
                                             mybir.ActivationFunctionType.Tanh)
                        oh1 = ohp.tile([128, CT_MAX, 128], BF16, tag="oh1")
                        nc.vector.tensor_tensor(out=oh1[:, 0:ct, :],
                                                in0=eqc[:, 0:ct, :], in1=th[:, 0:ct, :],
                                                op=mybir.AluOpType.mult)
                        ohc = ohp.tile([128, CT_MAX, 128], BF16, tag="ohc")
                        for (w, c0, c1) in runs:
                            rK = c1 - c0
                            dws = dinvw_sb[:, w, :]
                            dwx = bass.AP(dws.tensor, dws.offset,
                                          [list(dws.ap)[0], [0, rK], list(dws.ap)[1]])
                            nc.vector.tensor_tensor(out=ohc[:, c0:c1, :],
                                                    in0=oh1[:, c0:c1, :], in1=dwx,
                                                    op=mybir.AluOpType.mult)
                        # pass 2: one-hot scatter matmuls
                        for (w, c0, c1) in runs:
                            bf, bl = burst[(h, w)]
                            if t0 + c0 == bf:
                                W_ps = psW.tile([128, hidden], F32, tag="W")
                            for c in range(c0, c1):
                                nc.tensor.matmul(W_ps[:], ohc[:, c, :], G[:, c, 0:hh].bitcast(BF16),
                                                 start=(t0 + c == bf),
                                                 stop=(t0 + c == bl))
                            if t0 + c1 - 1 == bl:
                                if h == 0:
                                    nc.vector.tensor_tensor(out=acc[:, w, :], in0=W_ps[:],
                                                            in1=rawsc[:, w, :],
                                                            op=mybir.AluOpType.add)
                                    if last_stream[w] == 0:
                                        finalize(w)
                                else:
                                    nc.vector.tensor_tensor(out=acc[:, w, :], in0=W_ps[:],
                                                            in1=acc[:, w, :],
                                                            op=mybir.AluOpType.add)
                                    finalize(w)

            phase = os.environ.get("KPHASE", "head")
            plvl = {"prep": 0, "l0": 1, "cc": 2, "l1": 3, "head": 4}[phase]
            if plvl >= 1:
                nc.gpsimd.collective_compute(
                    "AllGather", mybir.AluOpType.bypass,
                    replica_groups=[list(range(ncores))],
                    ins=[agi0.ap().opt()], outs=[ext0.ap().opt()])
                emit_layer(0, ext0)
            if plvl >= 2:
                nc.gpsimd.collective_compute(
                    "AllGather", mybir.AluOpType.bypass,
                    replica_groups=[list(range(ncores))],
                    ins=[agi.ap().opt()], outs=[ago.ap().opt()])
            if plvl >= 3:
                emit_layer(1, ago)
            if plvl < 4:
                with tc.tile_pool(name="zout", bufs=1) as zp:
                    o_z = zp.tile([128, out_ch], F32, tag="oz")
                    nc.vector.memset(o_z[:], 0.0)
                    for w in range(nwin):
                        rows = 128 if w < nwin - 1 else last_win_rows
                        nc.sync.dma_start(out.ap()[w * 128:w * 128 + rows, :],
                                          o_z[0:rows, :])
                return nc

            # ---------------- head: out = log_softmax(h @ t2^T + b) ----------
            # two passes so the Act engine loads the Exp/Ln tables once each
            with tc.tile_pool(name="head", bufs=4) as hp, \
                 tc.tile_pool(name="head_ps", bufs=4, space="PSUM") as hps:
                o_all = cp.tile([128, nwin, out_ch], F32, tag="o_all")
                nm_all = cp.tile([128, nwin], F32, tag="nm_all")
                s_all = cp.tile([128, nwin], F32, tag="s_all")
                for w in range(nwin):
                    ht_ps = hps.tile([128, 128], F32, tag="ht")
                    nc.tensor.matmul(ht_ps[:], acc[:, w, :], ident_sb[:],
                                     start=True, stop=True)
                    ht_sb = hp.tile([128, 128], F32, tag="ht_sb")
                    nc.vector.tensor_copy(ht_sb[:], ht_ps[:])
                    o_ps = hps.tile([128, out_ch], F32, tag="ops")
                    nc.tensor.matmul(o_ps[:], ht_sb[:], t2wt_sb[:], start=True, stop=False)
                    nc.tensor.matmul(o_ps[:], ones_sb[:], t2b_sb[:], start=False, stop=True)
                    nc.vector.reduce_max(out=nm_all[:, w:w + 1], in_=o_ps[:],
                                         axis=mybir.AxisListType.X, negate=True)
                    e_sb = hp.tile([128, out_ch], F32, tag="e")
                    nc.scalar.activation(e_sb[:], o_ps[:],
                                         mybir.ActivationFunctionType.Exp,
                                         bias=nm_all[:, w:w + 1])
                    nc.vector.reduce_sum(out=s_all[:, w:w + 1], in_=e_sb[:],
                                         axis=mybir.AxisListType.X)
                    nc.vector.tensor_copy(o_all[:, w, :], o_ps[:])
                ls_all = cp.tile([128, nwin], F32, tag="ls_all")
                nc.scalar.activation(ls_all[:], s_all[:], mybir.ActivationFunctionType.Ln)
                for w in range(nwin):
                    rows = 128 if w < nwin - 1 else last_win_rows
                    o_sb = hp.tile([128, out_ch], F32, tag="o")
                    nc.vector.tensor_scalar(out=o_sb[:], in0=o_all[:, w, :],
                                            scalar1=nm_all[:, w:w + 1],
                                            scalar2=ls_all[:, w:w + 1],
                                            op0=mybir.AluOpType.add,
                                            op1=mybir.AluOpType.subtract)
                    nc.sync.dma_start(out.ap()[w * 128:w * 128 + rows, :], o_sb[0:rows, :])

    return nc


# ======================================================================
# Host driver
# ======================================================================

def _bf16(a):
    import ml_dtypes
    return np.asarray(a, dtype=ml_dtypes.bfloat16)


def _group_x(xT_pad, nrow_units, kt):
    # xT_pad: [in_ch+1, units*128] f32 -> xtg [ngrp, 128, PREP_GRP*kt, 128],
    # xbg [ngrp, 1, PREP_GRP, 128] (ones row)
    in_ch = (xT_pad.shape[0] - 1)
    ngrp_t = (nrow_units + PREP_GRP - 1) // PREP_GRP
    pad_units = ngrp_t * PREP_GRP
    xp = np.zeros((in_ch + 1, pad_units * 128), np.float32)
    xp[:, :xT_pad.shape[1]] = xT_pad
    # [in, u, 128] -> [u, in, 128]
    xr = xp[:in_ch].reshape(in_ch, pad_units, 128).transpose(1, 0, 2)
    # [g, c, k, p, r] with in = k*128+p
    xg = xr.reshape(ngrp_t, PREP_GRP, kt, 128, 128)
    xtg = np.ascontiguousarray(xg.transpose(0, 3, 1, 2, 4)).reshape(
        ngrp_t, 128, PREP_GRP * kt, 128)
    xb = xp[in_ch].reshape(ngrp_t, 1, PREP_GRP, 128)
    return _bf16(xtg), _bf16(np.ascontiguousarray(xb))


def kernel_run(x, edge_index, t1_w, t1_b, gate_w, gate_b, t2_w, t2_b,
               n_nodes=N_NODES, in_ch=IN_CH, hidden=HIDDEN, out_ch=OUT_CH,
               eps=EPS, ncores=NCORES, lo_split=None, trace=False):
    _install_profile_hook()
    from concourse import bass_utils

    if lo_split is None:
        lo_split = min(25000, ((n_nodes + 1) // 2 + 127) // 128 * 128)
    meta = preprocess(edge_index, n_nodes, ncores, lo_split)
    nwin = meta["nwin"]
    r_per = n_nodes // ncores
    kt = in_ch // 128
    dinv = meta["dinv"]

    nc = build_kernel(meta, n_nodes, in_ch, hidden, out_ch, eps, lo_split, ncores)
    nc.finalize()

    # host arrays
    x = np.asarray(x, np.float32)
    xT = np.concatenate([x.T, np.ones((1, x.shape[0]), np.float32)], axis=0)  # [in+1, N]

    t1wt_h = _bf16(np.concatenate([np.asarray(t1_w, np.float32).T,
                                   np.asarray(t1_b, np.float32)[None, :]], axis=0))
    gw = np.asarray(gate_w, np.float32)
    gwrep_h = _bf16(np.stack([
        np.tile(gw[0, :hidden][None, :], (128, 1)),
        np.tile(gw[0, hidden:][None, :], (128, 1)),
        np.tile(gw[1, :hidden][None, :], (128, 1)),
        np.tile(gw[1, hidden:][None, :], (128, 1))]))
    gbrep_h = np.tile(np.asarray(gate_b, np.float32)[None, :], (128, 1))
    t2wt_h = np.ascontiguousarray(np.asarray(t2_w, np.float32).T)
    t2b_h = np.asarray(t2_b, np.float32)[None, :]
    iotac_h = _bf16(np.tile(np.arange(128, dtype=np.float32)[None, :],
                            (128, CT_MAX)))
    ident_h = np.eye(128, dtype=np.float32)
    identb_h = _bf16(np.eye(128, dtype=np.float32))
    ones_h = np.ones((1, 128), np.float32)
    ones128b_h = _bf16(np.ones((128, 128), np.float32))

    dinv_pad = np.zeros(nwin * 128 * ncores + 128, np.float32)
    dinv_pad[:n_nodes] = dinv[:n_nodes] if dinv.shape[0] >= n_nodes else 0

    in_maps = []
    for c in range(ncores):
        sl = np.zeros((in_ch + 1, nwin * 128), np.float32)
        take = min(nwin * 128, xT.shape[1] - c * r_per)
        sl[:, :take] = xT[:, c * r_per: c * r_per + take]
        xtog_h, xbog_h = _group_x(sl, nwin, kt)
        # own-row dinv, padded to whole windows with zeros
        dv = np.zeros(nwin * 128, np.float32)
        dv[:min(r_per, nwin * 128)] = dinv[c * r_per:c * r_per + r_per]
        dinvw_h = _bf16(np.tile(dv[None, :], (128, 1)))
        dinvc_h = np.ascontiguousarray(dv.reshape(nwin, 128).T)  # [128, nwin]
        in_maps.append({
            "xtog": xtog_h, "xbog": xbog_h,
            "t1wt": t1wt_h, "gwrep": gwrep_h, "gbrep": gbrep_h,
            "t2wt": t2wt_h, "t2b": t2b_h,
            "iotac": iotac_h, "ident": ident_h, "identb": identb_h,
            "ones": ones_h, "ones128b": ones128b_h,
            "idx": meta["idx_dev"][c],
            "rrb": _bf16(meta["rr_dev"][c]),
            "dinvw": dinvw_h, "dinvc": dinvc_h,
        })

    res = bass_utils.run_bass_kernel_spmd(
        nc, in_maps, core_ids=list(range(ncores)), trace=trace)
    outp = np.concatenate([res.results[c]["out"] for c in range(ncores)], axis=0)
    return outp[:n_nodes], res


def kernel(**inputs):
    x = inputs["x"]
    edge_index = inputs["edge_index"]
    outp, _ = kernel_run(
        x, edge_index, inputs["t1_w"], inputs["t1_b"], inputs["gate_w"],
        inputs["gate_b"], inputs["t2_w"], inputs["t2_b"])
    return np.asarray(outp, np.float32)


# revision 27
# speedup vs baseline: 1.0649x; 1.0649x over previous
"""FAGCN forward on 8 TRN2 NeuronCores (Bass/Tile).

Sharding: row-partition of nodes, 8 ways. Each core projects only its own
rows (h = relu(x @ t1^T + b)) into 512B gather records
[dinv*h bf16 x128 | b bf16 replicated x128]; an AllGather assembles the
full table (same path builds the layer-1 table from finalized windows).
Per layer the edge phase is a two-stream token walk ([all-lo windows]
[all-hi windows], int16 gather indices split at lo_split): 1024-edge
SWDGE dma_gather calls fetch source records. The edge weight never
materializes per-token: the gate runs dest-expanded,
ohc[p,c,f] = (iota[f]==rr[p,c]) * tanh(awb[p,f] + brep[p,c,f]) * dinvw[f],
where every DVE operand is stride-1 in the innermost dim (2x mode), tanh
runs on the scalar engine, awb is the per-window a-vector broadcast built
by two small matmuls, and padding tokens carry rr=-1 so the equality mask
kills them. The scatter-add is a TensorE matmul per 128-token tile into a
per-window PSUM accumulator. The head (t2 matmul + log_softmax) is a
final two-pass sweep.
"""

import os
import sys
import numpy as np

sys.path.insert(0, "/opt/trn_rl_repo")

import concourse.bass as bass
import concourse.bacc as bacc
import concourse.mybir as mybir
import concourse.tile as tile
from concourse import library_config

F32 = mybir.dt.float32
BF16 = mybir.dt.bfloat16
I16 = mybir.dt.int16

# problem constants (self-contained per contract)
N_NODES = 50000
IN_CH = 256
HIDDEN = 128
OUT_CH = 64
EPS = 0.3
NCORES = 8
CALL_TOKENS = int(os.environ.get("KCT", "1024"))
CT_MAX = CALL_TOKENS // 128
EXT_SLOTS = 128   # 512B gather record
B_SLOT = 64       # first f32 slot of the bf16-replicated b region
PREP_GRP = 8


def _install_profile_hook():
    import types
    name = "antenv.axon_hooks"
    if name in sys.modules:
        return
    try:
        import trn_agent_boot.trn_boot as tb
        hook = tb._ntff_profile_via_ctypes("/opt/axon/libaxon_pjrt.so")
    except Exception:
        hook = None
    mod = types.ModuleType(name)
    mod._hook = hook
    mod.get_axon_ntff_profile_hook = lambda: mod._hook
    mod.set_axon_ntff_profile_hook = lambda h: setattr(mod, "_hook", h)
    sys.modules[name] = mod


# ======================================================================
# Host preprocessing: SPMD token streams + per-core data
# ======================================================================

def preprocess(edge_index, n_nodes, ncores, lo_split):
    row = np.asarray(edge_index[0], dtype=np.int64)
    col = np.asarray(edge_index[1], dtype=np.int64)
    E = row.shape[0]
    r_per = n_nodes // ncores
    nwin = (r_per + 127) // 128

    deg = np.bincount(row, minlength=n_nodes).astype(np.float64)
    dinv = np.where(deg > 0, 1.0 / np.sqrt(np.maximum(deg, 1.0)), 0.0).astype(np.float32)

    # A/B table split: table A = first wsplit windows of every core's rows,
    # table B = the rest. Keeps int16 row indices and lets the AllGather for
    # each half fire/pipeline independently.
    wsplit = (nwin + 1) // 2
    arows = min(wsplit * 128, r_per)
    brows = r_per - arows

    core = row // r_per
    lrow = row - core * r_per
    win = lrow // 128
    ccore = col // r_per
    lcol = col - ccore * r_per
    is_hi = (lcol >= arows).astype(np.int64)

    # stream order: core, then stream (A/B), then window, then lrow
    order = np.lexsort((lrow, win, is_hi, core))
    core_s, win_s, hi_s = core[order], win[order], is_hi[order]
    lrow_s = lrow[order]
    ccore_s, lcol_s = ccore[order], lcol[order]

    key = (core_s * 2 + hi_s) * nwin + win_s
    cnt = np.bincount(key, minlength=ncores * 2 * nwin).reshape(ncores, 2, nwin)
    sec_len = ((cnt.max(axis=0) + 127) // 128) * 128  # [2, nwin]
    L_lo = int(sec_len[0].sum())
    L_hi = int(sec_len[1].sum())
    e_tok = L_lo + L_hi
    sec_start = np.zeros((2, nwin), np.int64)
    sec_start[0] = np.concatenate([[0], np.cumsum(sec_len[0])[:-1]])
    sec_start[1] = L_lo + np.concatenate([[0], np.cumsum(sec_len[1])[:-1]])

    col16 = np.zeros((ncores, e_tok), np.int16)
    rowrel = np.full((ncores, e_tok), -1.0, np.float32)  # -1 = padding (mask)

    grp_first = np.zeros(ncores * 2 * nwin + 1, np.int64)
    np.cumsum(cnt.reshape(-1), out=grp_first[1:])
    rank = np.arange(E) - grp_first[key]
    dest = sec_start[hi_s, win_s] + rank
    cval = np.where(hi_s == 1, ccore_s * brows + (lcol_s - arows),
                    ccore_s * arows + lcol_s).astype(np.int16)
    col16[core_s, dest] = cval
    rowrel[core_s, dest] = (lrow_s - win_s * 128).astype(np.float32)

    # gather calls per stream
    calls = []  # (stream, ts, nt)
    for h, base, L in ((0, 0, L_lo), (1, L_lo, L_hi)):
        off = 0
        while off < L:
            nt = min(CALL_TOKENS, L - off)
            calls.append((h, base + off, nt))
            off += nt

    idx_dev = np.zeros((ncores, 128, e_tok // 16), np.int16)
    for (h, ts, nt) in calls:
        blk = col16[:, ts:ts + nt].reshape(ncores, nt // 16, 16)
        blk = np.ascontiguousarray(np.transpose(blk, (0, 2, 1)))
        idx_dev[:, :, ts // 16:(ts + nt) // 16] = np.tile(blk, (1, 8, 1))
    rr_dev = np.ascontiguousarray(rowrel.reshape(ncores, -1, 128).transpose(0, 2, 1))

    return {
        "nwin": nwin, "e_tok": e_tok, "sec_len": sec_len, "calls": calls,
        "idx_dev": idx_dev, "rr_dev": rr_dev, "dinv": dinv,
        "wsplit": wsplit, "arows": arows, "brows": brows,
    }


# ======================================================================
# Kernel builder
# ======================================================================

def build_kernel(meta, n_nodes, in_ch, hidden, out_ch, eps, lo_split, ncores):
    nwin = meta["nwin"]
    e_tok = meta["e_tok"]
    sec_len = meta["sec_len"]
    calls = meta["calls"]
    wsplit = meta["wsplit"]
    arows = meta["arows"]
    brows = meta["brows"]
    r_per = n_nodes // ncores
    last_win_rows = r_per - 128 * (nwin - 1)
    kt = in_ch // 128
    hh = hidden // 2  # f32 slots holding the bf16 h vector

    # tile -> window map, and burst boundaries per (stream, window)
    tiles_w = []
    burst = {}  # (h, w) -> (gfirst, glast) in global tile idx
    for h in range(2):
        for w in range(nwin):
            ntl = int(sec_len[h, w]) // 128
            if ntl == 0:
                continue
            g0 = len(tiles_w)
            tiles_w.extend([w] * ntl)
            burst[(h, w)] = (g0, g0 + ntl - 1)
    assert len(tiles_w) == e_tok // 128
    last_stream = {}
    for w in range(nwin):
        last_stream[w] = 1 if (1, w) in burst else 0

    ogrp = nwin // PREP_GRP
    ogrp_rem = nwin - ogrp * PREP_GRP

    nc = bacc.Bacc("TRN2", target_bir_lowering=False, debug=False,
                   num_devices=ncores, num_swdge_queues=4)

    # ---- I/O ----
    xtog = nc.dram_tensor("xtog", [ogrp + (1 if ogrp_rem else 0), 128, PREP_GRP * kt, 128], BF16, kind="ExternalInput")
    xbog = nc.dram_tensor("xbog", [ogrp + (1 if ogrp_rem else 0), 1, PREP_GRP, 128], BF16, kind="ExternalInput")
    t1wt = nc.dram_tensor("t1wt", [in_ch + 1, hidden], BF16, kind="ExternalInput")
    gwrep = nc.dram_tensor("gwrep", [4, 128, hidden], BF16, kind="ExternalInput")
    gbrep = nc.dram_tensor("gbrep", [128, 2], F32, kind="ExternalInput")
    t2wt = nc.dram_tensor("t2wt", [hidden, out_ch], F32, kind="ExternalInput")
    t2b = nc.dram_tensor("t2b", [1, out_ch], F32, kind="ExternalInput")
    iotac_in = nc.dram_tensor("iotac", [128, CT_MAX * 128], BF16, kind="ExternalInput")
    ident_in = nc.dram_tensor("ident", [128, 128], F32, kind="ExternalInput")
    identb_in = nc.dram_tensor("identb", [128, 128], BF16, kind="ExternalInput")
    ones_in = nc.dram_tensor("ones", [1, 128], F32, kind="ExternalInput")
    ones128b_in = nc.dram_tensor("ones128b", [128, 128], BF16, kind="ExternalInput")
    idx_in = nc.dram_tensor("idx", [128, e_tok // 16], I16, kind="ExternalInput")
    rrb_in = nc.dram_tensor("rrb", [128, e_tok // 128], BF16, kind="ExternalInput")
    dinvw_in = nc.dram_tensor("dinvw", [128, nwin * 128], BF16, kind="ExternalInput")
    dinvc_in = nc.dram_tensor("dinvc", [128, nwin], F32, kind="ExternalInput")
    out = nc.dram_tensor("out", [r_per, out_ch], F32, kind="ExternalOutput")

    ext0a = nc.dram_tensor("ext0a", [arows * ncores, EXT_SLOTS], F32, addr_space="Shared")
    ext0b = nc.dram_tensor("ext0b", [brows * ncores, EXT_SLOTS], F32, addr_space="Shared")
    agoa = nc.dram_tensor("agoa", [arows * ncores, EXT_SLOTS], F32, addr_space="Shared")
    agob = nc.dram_tensor("agob", [brows * ncores, EXT_SLOTS], F32, addr_space="Shared")
    agi0 = nc.dram_tensor("agi0", [r_per, EXT_SLOTS], F32)
    agi = nc.dram_tensor("agi", [r_per, EXT_SLOTS], F32)

    phase = os.environ.get("KPHASE", "head")
    plvl = {"prep": 0, "l0": 1, "cc": 2, "l1": 3, "head": 4}[phase]

    with tile.TileContext(nc) as tc:
        nc.gpsimd.load_library(library_config.mlp)
        with tc.tile_pool(name="consts", bufs=1) as cp:
            t1wt_sb = cp.tile([128, kt, hidden], BF16, tag="t1wt")
            nc.sync.dma_start(t1wt_sb[:], bass.AP(t1wt, 0, [[hidden, 128], [128 * hidden, kt], [1, hidden]]))
            t1b_sb = cp.tile([1, hidden], BF16, tag="t1b")
            nc.sync.dma_start(t1b_sb[:], t1wt.ap()[in_ch:in_ch + 1, :])
            gw_sb = cp.tile([128, 4, hidden], BF16, tag="gw")
            nc.sync.dma_start(gw_sb[:], bass.AP(gwrep, 0, [[hidden, 128], [128 * hidden, 4], [1, hidden]]))
            gb_sb = cp.tile([128, 2], F32, tag="gb")
            nc.sync.dma_start(gb_sb[:], gbrep.ap())
            t2wt_sb = cp.tile([128, out_ch], F32, tag="t2wt")
            nc.sync.dma_start(t2wt_sb[:], t2wt.ap())
            t2b_sb = cp.tile([1, out_ch], F32, tag="t2b")
            nc.sync.dma_start(t2b_sb[:], t2b.ap())
            ident_sb = cp.tile([128, 128], F32, tag="ident")
            nc.sync.dma_start(ident_sb[:], ident_in.ap())
            identb_sb = cp.tile([128, 128], BF16, tag="identb")
            nc.sync.dma_start(identb_sb[:], identb_in.ap())
            ones_sb = cp.tile([1, 128], F32, tag="ones")
            nc.sync.dma_start(ones_sb[:], ones_in.ap())
            ones128b_sb = cp.tile([128, 128], BF16, tag="ones128b")
            nc.sync.dma_start(ones128b_sb[:], ones128b_in.ap())
            idxt = cp.tile([128, e_tok // 16], I16, tag="idxt")
            nc.sync.dma_start(idxt[:], idx_in.ap())
            rrb_sb = cp.tile([128, e_tok // 128], BF16, tag="rrb")
            nc.sync.dma_start(rrb_sb[:], rrb_in.ap())
            dinvw_sb = cp.tile([128, nwin, 128], BF16, tag="dinvw")
            nc.sync.dma_start(dinvw_sb[:], dinvw_in.ap())
            dinvc_sb = cp.tile([128, nwin], F32, tag="dinvc")
            nc.sync.dma_start(dinvc_sb[:], dinvc_in.ap())
            iotac_sb = cp.tile([128, CT_MAX, 128], BF16, tag="iotac")
            nc.sync.dma_start(iotac_sb[:], iotac_in.ap())

            rawsc = cp.tile([128, nwin, hidden], F32, tag="rawsc")
            acc = cp.tile([128, nwin, hidden], F32, tag="acc")
            a_arr = cp.tile([128, nwin, 2], BF16, tag="a_arr")

            def cc_ag(in_ap, out_t):
                nc.gpsimd.collective_compute(
                    "AllGather", mybir.AluOpType.bypass,
                    replica_groups=[list(range(ncores))],
                    ins=[in_ap.opt()], outs=[out_t.ap().opt()])

            # ------- prep: own rows only -> records -> agi0 (AllGather later)
            with tc.tile_pool(name="prep", bufs=3) as pp, \
                 tc.tile_pool(name="prep_s", bufs=6) as pscr, \
                 tc.tile_pool(name="prep_ps", bufs=4, space="PSUM") as pps:

                def prep_group(gi, gcnt):
                    xt_sb = pp.tile([128, PREP_GRP * kt, 128], BF16, tag="xt")
                    nc.sync.dma_start(xt_sb[:, 0:gcnt * kt, :], xtog.ap()[gi, :, 0:gcnt * kt, :])
                    xb_sb = pp.tile([1, PREP_GRP, 128], BF16, tag="xb")
                    nc.sync.dma_start(xb_sb[:, 0:gcnt, :], xbog.ap()[gi, :, 0:gcnt, :])
                    for c in range(gcnt):
                        w = gi * PREP_GRP + c
                        rows = 128 if w < nwin - 1 else last_win_rows
                        ps = pps.tile([128, hidden], F32, tag="h0ps")
                        for k in range(kt):
                            nc.tensor.matmul(ps[:], xt_sb[:, c * kt + k, :], t1wt_sb[:, k, :],
                                             start=(k == 0), stop=False)
                        nc.tensor.matmul(ps[:], xb_sb[:, c, :], t1b_sb[:], start=False, stop=True)
                        nc.vector.tensor_scalar(out=rawsc[:, w, :], in0=ps[:],
                                                scalar1=0.0, scalar2=eps,
                                                op0=mybir.AluOpType.max,
                                                op1=mybir.AluOpType.mult)
                        extg = pp.tile([128, EXT_SLOTS], F32, tag="extg")
                        # record payload: dinv * relu(h)  (dinv >= 0 commutes with relu)
                        nc.scalar.activation(extg[:, 0:hh].bitcast(BF16), ps[:],
                                             mybir.ActivationFunctionType.Relu,
                                             scale=dinvc_sb[:, w:w + 1])
                        # gate terms from unscaled relu(h): STT does the relu via max
                        scr = pscr.tile([128, hidden], BF16, tag="scr")
                        a_f = pscr.tile([128, 1], F32, tag="af")
                        nc.vector.scalar_tensor_tensor(
                            out=scr[:], in0=ps[:], scalar=0.0, in1=gw_sb[:, 0, :],
                            op0=mybir.AluOpType.max, op1=mybir.AluOpType.mult,
                            accum_out=a_f[:])
                        nc.vector.tensor_scalar(out=a_arr[:, w, 0:1], in0=a_f[:],
                                                scalar1=gb_sb[:, 0:1], scalar2=None,
                                                op0=mybir.AluOpType.add)
                        scr2 = pscr.tile([128, hidden], BF16, tag="scr2")
                        b_f = pscr.tile([128, 1], F32, tag="bf")
                        nc.vector.scalar_tensor_tensor(
                            out=scr2[:], in0=ps[:], scalar=0.0, in1=gw_sb[:, 1, :],
                            op0=mybir.AluOpType.max, op1=mybir.AluOpType.mult,
                            accum_out=b_f[:])
                        nc.scalar.activation(extg[:, B_SLOT:].bitcast(BF16), ones128b_sb[:],
                                             mybir.ActivationFunctionType.Copy,
                                             scale=b_f[:])
                        nc.sync.dma_start(agi0.ap()[w * 128:w * 128 + rows, :],
                                          extg[0:rows, :])

                cc0a_done = False
                for gi in range(ogrp + (1 if ogrp_rem else 0)):
                    gcnt = PREP_GRP if gi < ogrp else ogrp_rem
                    prep_group(gi, gcnt)
                    if plvl >= 1 and not cc0a_done and gi * PREP_GRP + gcnt >= wsplit:
                        cc_ag(agi0.ap()[0:arows, :], ext0a)
                        cc0a_done = True
            if plvl >= 1:
                cc_ag(agi0.ap()[arows:r_per, :], ext0b)

            # ---------------- edge phase (per layer) ----------------
            def emit_layer(l, ta, tb, cc_mid=None, cc_end=None):
                lo_ap = ta.ap()
                hi_ap = tb.ap()
                # last B-stream tile that finalizes a window < wsplit: the
                # point after which the first-half AllGather can fire
                ccmid_tile = -1
                for w in range(min(wsplit, nwin)):
                    if (1, w) in burst:
                        ccmid_tile = max(ccmid_tile, burst[(1, w)][1])
                if ccmid_tile < 0:
                    ccmid_tile = int(sec_len[0].sum()) // 128 - 1
                awb = cp.tile([128, nwin, 128], BF16, tag=f"awb{l}")
                with tc.tile_pool(name=f"g{l}", bufs=int(os.environ.get("KGB", "4"))) as gp, \
                     tc.tile_pool(name=f"scr{l}", bufs=6) as scrp, \
                     tc.tile_pool(name=f"oh{l}", bufs=3) as ohp, \
                     tc.tile_pool(name=f"fin{l}", bufs=2) as fp, \
                     tc.tile_pool(name=f"aw{l}", bufs=4) as awp, \
                     tc.tile_pool(name=f"awps{l}", bufs=2, space="PSUM") as awps, \
                     tc.tile_pool(name=f"psW{l}", bufs=2, space="PSUM") as psW:
                    # per-window broadcast of the a-term: awb[p, w, f] = a_arr[f, w, l]
                    for w in range(nwin):
                        tp_ps = awps.tile([1, 128], F32, tag="aT")
                        nc.tensor.matmul(tp_ps[:], a_arr[:, w, l:l + 1], identb_sb[:],
                                         start=True, stop=True)
                        aT = awp.tile([1, 128], F32, tag="aTs")
                        nc.vector.tensor_copy(aT[:], tp_ps[:])
                        ab_ps = awps.tile([128, 128], F32, tag="ab")
                        nc.tensor.matmul(ab_ps[:], ones_sb[:], aT[:],
                                         start=True, stop=True)
                        nc.scalar.activation(awb[:, w, :], ab_ps[:],
                                             mybir.ActivationFunctionType.Copy)

                    def finalize(w):
                        rows = 128 if w < nwin - 1 else last_win_rows
                        if l == 0:
                            ext1 = fp.tile([128, EXT_SLOTS], F32, tag="ext1")
                            nc.scalar.activation(ext1[:, 0:hh].bitcast(BF16), acc[:, w, :],
                                                 mybir.ActivationFunctionType.Copy,
                                                 scale=dinvc_sb[:, w:w + 1])
                            scr = scrp.tile([128, hidden], BF16, tag="escr")
                            b_f = scrp.tile([128, 1], F32, tag="bf1")
                            nc.vector.scalar_tensor_tensor(
                                out=scr[:], in0=acc[:, w, :], scalar=1.0, in1=gw_sb[:, 3, :],
                                op0=mybir.AluOpType.mult, op1=mybir.AluOpType.mult,
                                accum_out=b_f[:])
                            nc.scalar.activation(ext1[:, B_SLOT:].bitcast(BF16),
                                                 ones128b_sb[:],
                                                 mybir.ActivationFunctionType.Copy,
                                                 scale=b_f[:])
                            scr2 = scrp.tile([128, hidden], BF16, tag="escr2")
                            a_f = scrp.tile([128, 1], F32, tag="af1")
                            nc.vector.scalar_tensor_tensor(
                                out=scr2[:], in0=acc[:, w, :], scalar=1.0, in1=gw_sb[:, 2, :],
                                op0=mybir.AluOpType.mult, op1=mybir.AluOpType.mult,
                                accum_out=a_f[:])
                            nc.vector.tensor_scalar(out=a_arr[:, w, 1:2], in0=a_f[:],
                                                    scalar1=gb_sb[:, 1:2], scalar2=None,
                                                    op0=mybir.AluOpType.add)
                            nc.sync.dma_start(agi.ap()[w * 128:w * 128 + rows, :],
                                              ext1[0:rows, :])

                    # windows with no lo-burst: seed acc with rawsc; fully
                    # edgeless windows also finalize immediately
                    for w in range(nwin):
                        if (0, w) not in burst:
                            nc.vector.tensor_copy(acc[:, w, :], rawsc[:, w, :])
                            if (1, w) not in burst:
                                finalize(w)

                    qi = 0
                    W_ps = None
                    for (h, ts, nt) in [c for c in calls]:
                        ct = nt // 128
                        t0 = ts // 128
                        G = gp.tile([128, CT_MAX, EXT_SLOTS], F32, tag="G")
                        nc.gpsimd.dma_gather(
                            out_ap=G[:, 0:ct, :],
                            in_ap=(hi_ap if h else lo_ap),
                            idxs_ap=idxt[:, ts // 16:(ts + nt) // 16],
                            num_idxs=nt, num_idxs_reg=nt, elem_size=EXT_SLOTS,
                            single_packet=False, queue_num=qi % 4)
                        qi += 1
                        # split call into window-pure runs
                        runs = []
                        c = 0
                        while c < ct:
                            w = tiles_w[t0 + c]
                            c1 = c
                            while c1 < ct and tiles_w[t0 + c1] == w:
                                c1 += 1
                            runs.append((w, c, c1))
                            c = c1
                        # mask: eqc[p, c, f] = (iota[f]==rr[p,c]); rr=-1 pads
                        eqc = ohp.tile([128, CT_MAX, 128], BF16, tag="eqc")
                        rrs = rrb_sb[:, t0:t0 + ct]
                        rrx = bass.AP(rrs.tensor, rrs.offset, list(rrs.ap) + [[0, 128]])
                        nc.vector.tensor_tensor(out=eqc[:, 0:ct, :],
                                                in0=iotac_sb[:, 0:ct, :], in1=rrx,
                                                op=mybir.AluOpType.is_equal)
                        # dest-expanded gate arg: s1 = awb[p,f] + b[p,c]
                        s1 = ohp.tile([128, CT_MAX, 128], BF16, tag="s1")
                        for (w, c0, c1) in runs:
                            rK = c1 - c0
                            aws = awb[:, w, :]
                            awx = bass.AP(aws.tensor, aws.offset,
                                          [list(aws.ap)[0], [0, rK], list(aws.ap)[1]])
                            bx = G[:, c0:c1, B_SLOT:].bitcast(BF16)
                            nc.vector.tensor_tensor(out=s1[:, c0:c1, :],
                                                    in0=bx, in1=awx,
                                                    op=mybir.AluOpType.add)
                        th = ohp.tile([128, CT_MAX, 128], BF16, tag="th")
                        nc.scalar.activation(th[:, 0:ct, :], s1[:, 0:ct, :],
                                             mybir.ActivationFunctionType.Tanh)
                        oh1 = ohp.tile([128, CT_MAX, 128], BF16, tag="oh1")
                        nc.vector.tensor_tensor(out=oh1[:, 0:ct, :],
                                                in0=eqc[:, 0:ct, :], in1=th[:, 0:ct, :],
                                                op=mybir.AluOpType.mult)
                        ohc = ohp.tile([128, CT_MAX, 128], BF16, tag="ohc")
                        for (w, c0, c1) in runs:
                            rK = c1 - c0
                            dws = dinvw_sb[:, w, :]
                            dwx = bass.AP(dws.tensor, dws.offset,
                                          [list(dws.ap)[0], [0, rK], list(dws.ap)[1]])
                            nc.vector.tensor_tensor(out=ohc[:, c0:c1, :],
                                                    in0=oh1[:, c0:c1, :], in1=dwx,
                                                    op=mybir.AluOpType.mult)
                        # pass 2: one-hot scatter matmuls
                        for (w, c0, c1) in runs:
                            bf, bl = burst[(h, w)]
                            if t0 + c0 == bf:
                                W_ps = psW.tile([128, hidden], F32, tag="W")
                            for c in range(c0, c1):
                                nc.tensor.matmul(W_ps[:], ohc[:, c, :], G[:, c, 0:hh].bitcast(BF16),
                                                 start=(t0 + c == bf),
                                                 stop=(t0 + c == bl))
                            if t0 + c1 - 1 == bl:
                                if h == 0:
                                    nc.vector.tensor_tensor(out=acc[:, w, :], in0=W_ps[:],
                                                            in1=rawsc[:, w, :],
                                                            op=mybir.AluOpType.add)
                                    if last_stream[w] == 0:
                                        finalize(w)
                                else:
                                    nc.vector.tensor_tensor(out=acc[:, w, :], in0=W_ps[:],
                                                            in1=acc[:, w, :],
                                                            op=mybir.AluOpType.add)
                                    finalize(w)
                        if cc_mid is not None and t0 <= ccmid_tile < t0 + ct:
                            cc_mid()
                    if cc_end is not None:
                        cc_end()

            if plvl >= 1:
                emit_layer(
                    0, ext0a, ext0b,
                    cc_mid=(lambda: cc_ag(agi.ap()[0:arows, :], agoa))
                    if plvl >= 2 else None,
                    cc_end=(lambda: cc_ag(agi.ap()[arows:r_per, :], agob))
                    if plvl >= 2 else None)
            if plvl >= 3:
                emit_layer(1, agoa, agob)
            if plvl < 4:
                with tc.tile_pool(name="zout", bufs=1) as zp:
                    o_z = zp.tile([128, out_ch], F32, tag="oz")
                    nc.vector.memset(o_z[:], 0.0)
                    for w in range(nwin):
                        rows = 128 if w < nwin - 1 else last_win_rows
                        nc.sync.dma_start(out.ap()[w * 128:w * 128 + rows, :],
                                          o_z[0:rows, :])
                return nc

            # ---------------- head: out = log_softmax(h @ t2^T + b) ----------
            # two passes so the Act engine loads the Exp/Ln tables once each
            with tc.tile_pool(name="head", bufs=4) as hp, \
                 tc.tile_pool(name="head_ps", bufs=4, space="PSUM") as hps:
                o_all = cp.tile([128, nwin, out_ch], F32, tag="o_all")
                nm_all = cp.tile([128, nwin], F32, tag="nm_all")
                s_all = cp.tile([128, nwin], F32, tag="s_all")
                for w in range(nwin):
                    ht_ps = hps.tile([128, 128], F32, tag="ht")
                    nc.tensor.matmul(ht_ps[:], acc[:, w, :], ident_sb[:],
                                     start=True, stop=True)
                    ht_sb = hp.tile([128, 128], F32, tag="ht_sb")
                    nc.vector.tensor_copy(ht_sb[:], ht_ps[:])
                    o_ps = hps.tile([128, out_ch], F32, tag="ops")
                    nc.tensor.matmul(o_ps[:], ht_sb[:], t2wt_sb[:], start=True, stop=False)
                    nc.tensor.matmul(o_ps[:], ones_sb[:], t2b_sb[:], start=False, stop=True)
                    nc.vector.reduce_max(out=nm_all[:, w:w + 1], in_=o_ps[:],
                                         axis=mybir.AxisListType.X, negate=True)
                    e_sb = hp.tile([128, out_ch], F32, tag="e")
                    nc.scalar.activation(e_sb[:], o_ps[:],
                                         mybir.ActivationFunctionType.Exp,
                                         bias=nm_all[:, w:w + 1])
                    nc.vector.reduce_sum(out=s_all[:, w:w + 1], in_=e_sb[:],
                                         axis=mybir.AxisListType.X)
                    nc.vector.tensor_copy(o_all[:, w, :], o_ps[:])
                ls_all = cp.tile([128, nwin], F32, tag="ls_all")
                nc.scalar.activation(ls_all[:], s_all[:], mybir.ActivationFunctionType.Ln)
                for w in range(nwin):
                    rows = 128 if w < nwin - 1 else last_win_rows
                    o_sb = hp.tile([128, out_ch], F32, tag="o")
                    nc.vector.tensor_scalar(out=o_sb[:], in0=o_all[:, w, :],
                                            scalar1=nm_all[:, w:w + 1],
                                            scalar2=ls_all[:, w:w + 1],
                                            op0=mybir.AluOpType.add,
                                            op1=mybir.AluOpType.subtract)
                    nc.sync.dma_start(out.ap()[w * 128:w * 128 + rows, :], o_sb[0:rows, :])

    return nc


# ======================================================================
# Host driver
# ======================================================================

def _bf16(a):
    import ml_dtypes
    return np.asarray(a, dtype=ml_dtypes.bfloat16)


def _group_x(xT_pad, nrow_units, kt):
    # xT_pad: [in_ch+1, units*128] f32 -> xtg [ngrp, 128, PREP_GRP*kt, 128],
    # xbg [ngrp, 1, PREP_GRP, 128] (ones row)
    in_ch = (xT_pad.shape[0] - 1)
    ngrp_t = (nrow_units + PREP_GRP - 1) // PREP_GRP
    pad_units = ngrp_t * PREP_GRP
    xp = np.zeros((in_ch + 1, pad_units * 128), np.float32)
    xp[:, :xT_pad.shape[1]] = xT_pad
    # [in, u, 128] -> [u, in, 128]
    xr = xp[:in_ch].reshape(in_ch, pad_units, 128).transpose(1, 0, 2)
    # [g, c, k, p, r] with in = k*128+p
    xg = xr.reshape(ngrp_t, PREP_GRP, kt, 128, 128)
    xtg = np.ascontiguousarray(xg.transpose(0, 3, 1, 2, 4)).reshape(
        ngrp_t, 128, PREP_GRP * kt, 128)
    xb = xp[in_ch].reshape(ngrp_t, 1, PREP_GRP, 128)
    return _bf16(xtg), _bf16(np.ascontiguousarray(xb))


def kernel_run(x, edge_index, t1_w, t1_b, gate_w, gate_b, t2_w, t2_b,
               n_nodes=N_NODES, in_ch=IN_CH, hidden=HIDDEN, out_ch=OUT_CH,
               eps=EPS, ncores=NCORES, lo_split=None, trace=False):
    _install_profile_hook()
    from concourse import bass_utils

    if lo_split is None:
        lo_split = min(25000, ((n_nodes + 1) // 2 + 127) // 128 * 128)
    meta = preprocess(edge_index, n_nodes, ncores, lo_split)
    nwin = meta["nwin"]
    r_per = n_nodes // ncores
    kt = in_ch // 128
    dinv = meta["dinv"]

    nc = build_kernel(meta, n_nodes, in_ch, hidden, out_ch, eps, lo_split, ncores)
    nc.finalize()

    # host arrays
    x = np.asarray(x, np.float32)
    xT = np.concatenate([x.T, np.ones((1, x.shape[0]), np.float32)], axis=0)  # [in+1, N]

    t1wt_h = _bf16(np.concatenate([np.asarray(t1_w, np.float32).T,
                                   np.asarray(t1_b, np.float32)[None, :]], axis=0))
    gw = np.asarray(gate_w, np.float32)
    gwrep_h = _bf16(np.stack([
        np.tile(gw[0, :hidden][None, :], (128, 1)),
        np.tile(gw[0, hidden:][None, :], (128, 1)),
        np.tile(gw[1, :hidden][None, :], (128, 1)),
        np.tile(gw[1, hidden:][None, :], (128, 1))]))
    gbrep_h = np.tile(np.asarray(gate_b, np.float32)[None, :], (128, 1))
    t2wt_h = np.ascontiguousarray(np.asarray(t2_w, np.float32).T)
    t2b_h = np.asarray(t2_b, np.float32)[None, :]
    iotac_h = _bf16(np.tile(np.arange(128, dtype=np.float32)[None, :],
                            (128, CT_MAX)))
    ident_h = np.eye(128, dtype=np.float32)
    identb_h = _bf16(np.eye(128, dtype=np.float32))
    ones_h = np.ones((1, 128), np.float32)
    ones128b_h = _bf16(np.ones((128, 128), np.float32))

    dinv_pad = np.zeros(nwin * 128 * ncores + 128, np.float32)
    dinv_pad[:n_nodes] = dinv[:n_nodes] if dinv.shape[0] >= n_nodes else 0

    in_maps = []
    for c in range(ncores):
        sl = np.zeros((in_ch + 1, nwin * 128), np.float32)
        take = min(nwin * 128, xT.shape[1] - c * r_per)
        sl[:, :take] = xT[:, c * r_per: c * r_per + take]
        xtog_h, xbog_h = _group_x(sl, nwin, kt)
        # own-row dinv, padded to whole windows with zeros
        dv = np.zeros(nwin * 128, np.float32)
        dv[:min(r_per, nwin * 128)] = dinv[c * r_per:c * r_per + r_per]
        dinvw_h = _bf16(np.tile(dv[None, :], (128, 1)))
        dinvc_h = np.ascontiguousarray(dv.reshape(nwin, 128).T)  # [128, nwin]
        in_maps.append({
            "xtog": xtog_h, "xbog": xbog_h,
            "t1wt": t1wt_h, "gwrep": gwrep_h, "gbrep": gbrep_h,
            "t2wt": t2wt_h, "t2b": t2b_h,
            "iotac": iotac_h, "ident": ident_h, "identb": identb_h,
            "ones": ones_h, "ones128b": ones128b_h,
            "idx": meta["idx_dev"][c],
            "rrb": _bf16(meta["rr_dev"][c]),
            "dinvw": dinvw_h, "dinvc": dinvc_h,
        })

    res = bass_utils.run_bass_kernel_spmd(
        nc, in_maps, core_ids=list(range(ncores)), trace=trace)
    outp = np.concatenate([res.results[c]["out"] for c in range(ncores)], axis=0)
    return outp[:n_nodes], res


def kernel(**inputs):
    x = inputs["x"]
    edge_index = inputs["edge_index"]
    outp, _ = kernel_run(
        x, edge_index, inputs["t1_w"], inputs["t1_b"], inputs["gate_w"],
        inputs["gate_b"], inputs["t2_w"], inputs["t2_b"])
    return np.asarray(outp, np.float32)


# revision 29
# speedup vs baseline: 1.1213x; 1.0529x over previous
"""FAGCN forward on 8 TRN2 NeuronCores (Bass/Tile).

Sharding: row-partition of nodes, 8 ways. Each core projects only its own
rows (h = relu(x @ t1^T + b)) into 512B gather records
[dinv*h bf16 x128 | b bf16 replicated x128]; an AllGather assembles the
full table (same path builds the layer-1 table from finalized windows).
Per layer the edge phase is a two-stream token walk ([all-lo windows]
[all-hi windows], int16 gather indices split at lo_split): 1024-edge
SWDGE dma_gather calls fetch source records. The edge weight never
materializes per-token: the gate runs dest-expanded,
ohc[p,c,f] = (iota[f]==rr[p,c]) * tanh(awb[p,f] + brep[p,c,f]) * dinvw[f],
where every DVE operand is stride-1 in the innermost dim (2x mode), tanh
runs on the scalar engine, awb is the per-window a-vector broadcast built
by two small matmuls, and padding tokens carry rr=-1 so the equality mask
kills them. The scatter-add is a TensorE matmul per 128-token tile into a
per-window PSUM accumulator. The head (t2 matmul + log_softmax) is a
final two-pass sweep.
"""

import os
import sys
import numpy as np

sys.path.insert(0, "/opt/trn_rl_repo")

import concourse.bass as bass
import concourse.bacc as bacc
import concourse.mybir as mybir
import concourse.tile as tile
from concourse import library_config

F32 = mybir.dt.float32
BF16 = mybir.dt.bfloat16
FP8 = mybir.dt.float8e4
I16 = mybir.dt.int16

# problem constants (self-contained per contract)
N_NODES = 50000
IN_CH = 256
HIDDEN = 128
OUT_CH = 64
EPS = 0.3
NCORES = 8
CALL_TOKENS = int(os.environ.get("KCT", "1024"))
CT_MAX = CALL_TOKENS // 128
EXT_SLOTS = 64    # 256B gather record [h fp8 x128 | b bf16 x64]
B_SLOT = 32       # first f32 slot of the bf16-replicated b region
PREP_GRP = 8


def _install_profile_hook():
    import types
    name = "antenv.axon_hooks"
    if name in sys.modules:
        return
    try:
        import trn_agent_boot.trn_boot as tb
        hook = tb._ntff_profile_via_ctypes("/opt/axon/libaxon_pjrt.so")
    except Exception:
        hook = None
    mod = types.ModuleType(name)
    mod._hook = hook
    mod.get_axon_ntff_profile_hook = lambda: mod._hook
    mod.set_axon_ntff_profile_hook = lambda h: setattr(mod, "_hook", h)
    sys.modules[name] = mod


# ======================================================================
# Host preprocessing: SPMD token streams + per-core data
# ======================================================================

def preprocess(edge_index, n_nodes, ncores, lo_split):
    row = np.asarray(edge_index[0], dtype=np.int64)
    col = np.asarray(edge_index[1], dtype=np.int64)
    E = row.shape[0]
    r_per = n_nodes // ncores
    nwin = (r_per + 127) // 128

    deg = np.bincount(row, minlength=n_nodes).astype(np.float64)
    dinv = np.where(deg > 0, 1.0 / np.sqrt(np.maximum(deg, 1.0)), 0.0).astype(np.float32)

    # A/B table split: table A = first wsplit windows of every core's rows,
    # table B = the rest. Keeps int16 row indices and lets the AllGather for
    # each half fire/pipeline independently.
    wsplit = (nwin + 1) // 2
    arows = min(wsplit * 128, r_per)
    brows = r_per - arows

    core = row // r_per
    lrow = row - core * r_per
    win = lrow // 128
    ccore = col // r_per
    lcol = col - ccore * r_per
    is_hi = (lcol >= arows).astype(np.int64)

    # stream order: core, then stream (A/B), then window, then lrow
    order = np.lexsort((lrow, win, is_hi, core))
    core_s, win_s, hi_s = core[order], win[order], is_hi[order]
    lrow_s = lrow[order]
    ccore_s, lcol_s = ccore[order], lcol[order]

    key = (core_s * 2 + hi_s) * nwin + win_s
    cnt = np.bincount(key, minlength=ncores * 2 * nwin).reshape(ncores, 2, nwin)
    sec_len = ((cnt.max(axis=0) + 127) // 128) * 128  # [2, nwin]
    L_lo = int(sec_len[0].sum())
    L_hi = int(sec_len[1].sum())
    e_tok = L_lo + L_hi
    sec_start = np.zeros((2, nwin), np.int64)
    sec_start[0] = np.concatenate([[0], np.cumsum(sec_len[0])[:-1]])
    sec_start[1] = L_lo + np.concatenate([[0], np.cumsum(sec_len[1])[:-1]])

    col16 = np.zeros((ncores, e_tok), np.int16)
    rowrel = np.full((ncores, e_tok), -1.0, np.float32)  # -1 = padding (mask)

    grp_first = np.zeros(ncores * 2 * nwin + 1, np.int64)
    np.cumsum(cnt.reshape(-1), out=grp_first[1:])
    rank = np.arange(E) - grp_first[key]
    dest = sec_start[hi_s, win_s] + rank
    cval = np.where(hi_s == 1, ccore_s * brows + (lcol_s - arows),
                    ccore_s * arows + lcol_s).astype(np.int16)
    col16[core_s, dest] = cval
    rowrel[core_s, dest] = (lrow_s - win_s * 128).astype(np.float32)

    # gather calls per stream
    calls = []  # (stream, ts, nt)
    for h, base, L in ((0, 0, L_lo), (1, L_lo, L_hi)):
        off = 0
        while off < L:
            nt = min(CALL_TOKENS, L - off)
            calls.append((h, base + off, nt))
            off += nt

    idx_dev = np.zeros((ncores, 128, e_tok // 16), np.int16)
    for (h, ts, nt) in calls:
        blk = col16[:, ts:ts + nt].reshape(ncores, nt // 16, 16)
        blk = np.ascontiguousarray(np.transpose(blk, (0, 2, 1)))
        idx_dev[:, :, ts // 16:(ts + nt) // 16] = np.tile(blk, (1, 8, 1))
    rr_dev = np.ascontiguousarray(rowrel.reshape(ncores, -1, 128).transpose(0, 2, 1))

    return {
        "nwin": nwin, "e_tok": e_tok, "sec_len": sec_len, "calls": calls,
        "idx_dev": idx_dev, "rr_dev": rr_dev, "dinv": dinv,
        "wsplit": wsplit, "arows": arows, "brows": brows,
    }


# ======================================================================
# Kernel builder
# ======================================================================

def build_kernel(meta, n_nodes, in_ch, hidden, out_ch, eps, lo_split, ncores):
    nwin = meta["nwin"]
    e_tok = meta["e_tok"]
    sec_len = meta["sec_len"]
    calls = meta["calls"]
    wsplit = meta["wsplit"]
    arows = meta["arows"]
    brows = meta["brows"]
    r_per = n_nodes // ncores
    last_win_rows = r_per - 128 * (nwin - 1)
    kt = in_ch // 128
    hq = hidden // 4  # f32 slots holding the fp8 h vector

    # tile -> window map, and burst boundaries per (stream, window)
    tiles_w = []
    burst = {}  # (h, w) -> (gfirst, glast) in global tile idx
    for h in range(2):
        for w in range(nwin):
            ntl = int(sec_len[h, w]) // 128
            if ntl == 0:
                continue
            g0 = len(tiles_w)
            tiles_w.extend([w] * ntl)
            burst[(h, w)] = (g0, g0 + ntl - 1)
    assert len(tiles_w) == e_tok // 128
    last_stream = {}
    for w in range(nwin):
        last_stream[w] = 1 if (1, w) in burst else 0

    ogrp = nwin // PREP_GRP
    ogrp_rem = nwin - ogrp * PREP_GRP

    nc = bacc.Bacc("TRN2", target_bir_lowering=False, debug=False,
                   num_devices=ncores, num_swdge_queues=4)

    # ---- I/O ----
    xtog = nc.dram_tensor("xtog", [ogrp + (1 if ogrp_rem else 0), 128, PREP_GRP * kt, 128], BF16, kind="ExternalInput")
    xbog = nc.dram_tensor("xbog", [ogrp + (1 if ogrp_rem else 0), 1, PREP_GRP, 128], BF16, kind="ExternalInput")
    t1wt = nc.dram_tensor("t1wt", [in_ch + 1, hidden], BF16, kind="ExternalInput")
    gwrep = nc.dram_tensor("gwrep", [4, 128, hidden], BF16, kind="ExternalInput")
    gbrep = nc.dram_tensor("gbrep", [128, 2], F32, kind="ExternalInput")
    t2wt = nc.dram_tensor("t2wt", [hidden, out_ch], F32, kind="ExternalInput")
    t2b = nc.dram_tensor("t2b", [1, out_ch], F32, kind="ExternalInput")
    iotac_in = nc.dram_tensor("iotac", [128, CT_MAX * 128], BF16, kind="ExternalInput")
    ident_in = nc.dram_tensor("ident", [128, 128], F32, kind="ExternalInput")
    identb_in = nc.dram_tensor("identb", [128, 128], BF16, kind="ExternalInput")
    ones_in = nc.dram_tensor("ones", [1, 128], F32, kind="ExternalInput")
    ones128b_in = nc.dram_tensor("ones128b", [128, 128], BF16, kind="ExternalInput")
    idx_in = nc.dram_tensor("idx", [128, e_tok // 16], I16, kind="ExternalInput")
    rrb_in = nc.dram_tensor("rrb", [128, e_tok // 128], BF16, kind="ExternalInput")
    dinvw_in = nc.dram_tensor("dinvw", [128, nwin * 128], BF16, kind="ExternalInput")
    dinvc_in = nc.dram_tensor("dinvc", [128, nwin], F32, kind="ExternalInput")
    out = nc.dram_tensor("out", [r_per, out_ch], F32, kind="ExternalOutput")

    ext0a = nc.dram_tensor("ext0a", [arows * ncores, EXT_SLOTS], F32, addr_space="Shared")
    ext0b = nc.dram_tensor("ext0b", [brows * ncores, EXT_SLOTS], F32, addr_space="Shared")
    agoa = nc.dram_tensor("agoa", [arows * ncores, EXT_SLOTS], F32, addr_space="Shared")
    agob = nc.dram_tensor("agob", [brows * ncores, EXT_SLOTS], F32, addr_space="Shared")
    agi0 = nc.dram_tensor("agi0", [r_per, EXT_SLOTS], F32)
    agi = nc.dram_tensor("agi", [r_per, EXT_SLOTS], F32)

    phase = os.environ.get("KPHASE", "head")
    plvl = {"prep": 0, "l0": 1, "cc": 2, "l1": 3, "head": 4}[phase]

    with tile.TileContext(nc) as tc:
        nc.gpsimd.load_library(library_config.mlp)
        with tc.tile_pool(name="consts", bufs=1) as cp:
            t1wt_sb = cp.tile([128, kt, hidden], BF16, tag="t1wt")
            nc.sync.dma_start(t1wt_sb[:], bass.AP(t1wt, 0, [[hidden, 128], [128 * hidden, kt], [1, hidden]]))
            t1b_sb = cp.tile([1, hidden], BF16, tag="t1b")
            nc.sync.dma_start(t1b_sb[:], t1wt.ap()[in_ch:in_ch + 1, :])
            gw_sb = cp.tile([128, 4, hidden], BF16, tag="gw")
            nc.sync.dma_start(gw_sb[:], bass.AP(gwrep, 0, [[hidden, 128], [128 * hidden, 4], [1, hidden]]))
            gb_sb = cp.tile([128, 2], F32, tag="gb")
            nc.sync.dma_start(gb_sb[:], gbrep.ap())
            t2wt_sb = cp.tile([128, out_ch], F32, tag="t2wt")
            nc.sync.dma_start(t2wt_sb[:], t2wt.ap())
            t2b_sb = cp.tile([1, out_ch], F32, tag="t2b")
            nc.sync.dma_start(t2b_sb[:], t2b.ap())
            ident_sb = cp.tile([128, 128], F32, tag="ident")
            nc.sync.dma_start(ident_sb[:], ident_in.ap())
            identb_sb = cp.tile([128, 128], BF16, tag="identb")
            nc.sync.dma_start(identb_sb[:], identb_in.ap())
            ones_sb = cp.tile([1, 128], F32, tag="ones")
            nc.sync.dma_start(ones_sb[:], ones_in.ap())
            ones128b_sb = cp.tile([128, 128], BF16, tag="ones128b")
            nc.sync.dma_start(ones128b_sb[:], ones128b_in.ap())
            idxt = cp.tile([128, e_tok // 16], I16, tag="idxt")
            nc.sync.dma_start(idxt[:], idx_in.ap())
            rrb_sb = cp.tile([128, e_tok // 128], BF16, tag="rrb")
            nc.sync.dma_start(rrb_sb[:], rrb_in.ap())
            dinvw_sb = cp.tile([128, nwin, 128], BF16, tag="dinvw")
            nc.sync.dma_start(dinvw_sb[:], dinvw_in.ap())
            dinvc_sb = cp.tile([128, nwin], F32, tag="dinvc")
            nc.sync.dma_start(dinvc_sb[:], dinvc_in.ap())
            iotac_sb = cp.tile([128, CT_MAX, 128], BF16, tag="iotac")
            nc.sync.dma_start(iotac_sb[:], iotac_in.ap())

            rawsc = cp.tile([128, nwin, hidden], F32, tag="rawsc")
            acc = cp.tile([128, nwin, hidden], F32, tag="acc")
            a_arr = cp.tile([128, nwin, 2], BF16, tag="a_arr")

            def cc_ag(in_ap, out_t):
                nc.gpsimd.collective_compute(
                    "AllGather", mybir.AluOpType.bypass,
                    replica_groups=[list(range(ncores))],
                    ins=[in_ap.opt()], outs=[out_t.ap().opt()])

            # ------- prep: own rows only -> records -> agi0 (AllGather later)
            with tc.tile_pool(name="prep", bufs=3) as pp, \
                 tc.tile_pool(name="prep_s", bufs=6) as pscr, \
                 tc.tile_pool(name="prep_ps", bufs=4, space="PSUM") as pps:

                def prep_group(gi, gcnt):
                    xt_sb = pp.tile([128, PREP_GRP * kt, 128], BF16, tag="xt")
                    nc.sync.dma_start(xt_sb[:, 0:gcnt * kt, :], xtog.ap()[gi, :, 0:gcnt * kt, :])
                    xb_sb = pp.tile([1, PREP_GRP, 128], BF16, tag="xb")
                    nc.sync.dma_start(xb_sb[:, 0:gcnt, :], xbog.ap()[gi, :, 0:gcnt, :])
                    for c in range(gcnt):
                        w = gi * PREP_GRP + c
                        rows = 128 if w < nwin - 1 else last_win_rows
                        ps = pps.tile([128, hidden], F32, tag="h0ps")
                        for k in range(kt):
                            nc.tensor.matmul(ps[:], xt_sb[:, c * kt + k, :], t1wt_sb[:, k, :],
                                             start=(k == 0), stop=False)
                        nc.tensor.matmul(ps[:], xb_sb[:, c, :], t1b_sb[:], start=False, stop=True)
                        nc.vector.tensor_scalar(out=rawsc[:, w, :], in0=ps[:],
                                                scalar1=0.0, scalar2=eps,
                                                op0=mybir.AluOpType.max,
                                                op1=mybir.AluOpType.mult)
                        extg = pp.tile([128, EXT_SLOTS], F32, tag="extg")
                        # record payload: dinv * relu(h)  (dinv >= 0 commutes with relu)
                        nc.scalar.activation(extg[:, 0:hq].bitcast(FP8), ps[:],
                                             mybir.ActivationFunctionType.Relu,
                                             scale=dinvc_sb[:, w:w + 1])
                        # gate terms from unscaled relu(h): STT does the relu via max
                        scr = pscr.tile([128, hidden], BF16, tag="scr")
                        a_f = pscr.tile([128, 1], F32, tag="af")
                        nc.vector.scalar_tensor_tensor(
                            out=scr[:], in0=ps[:], scalar=0.0, in1=gw_sb[:, 0, :],
                            op0=mybir.AluOpType.max, op1=mybir.AluOpType.mult,
                            accum_out=a_f[:])
                        nc.vector.tensor_scalar(out=a_arr[:, w, 0:1], in0=a_f[:],
                                                scalar1=gb_sb[:, 0:1], scalar2=None,
                                                op0=mybir.AluOpType.add)
                        scr2 = pscr.tile([128, hidden], BF16, tag="scr2")
                        b_f = pscr.tile([128, 1], F32, tag="bf")
                        nc.vector.scalar_tensor_tensor(
                            out=scr2[:], in0=ps[:], scalar=0.0, in1=gw_sb[:, 1, :],
                            op0=mybir.AluOpType.max, op1=mybir.AluOpType.mult,
                            accum_out=b_f[:])
                        nc.scalar.activation(extg[:, B_SLOT:].bitcast(BF16),
                                             ones128b_sb[:, 0:64],
                                             mybir.ActivationFunctionType.Copy,
                                             scale=b_f[:])
                        nc.sync.dma_start(agi0.ap()[w * 128:w * 128 + rows, :],
                                          extg[0:rows, :])

                cc0a_done = False
                for gi in range(ogrp + (1 if ogrp_rem else 0)):
                    gcnt = PREP_GRP if gi < ogrp else ogrp_rem
                    prep_group(gi, gcnt)
                    if plvl >= 1 and not cc0a_done and gi * PREP_GRP + gcnt >= wsplit:
                        cc_ag(agi0.ap()[0:arows, :], ext0a)
                        cc0a_done = True
            if plvl >= 1:
                cc_ag(agi0.ap()[arows:r_per, :], ext0b)

            # ---------------- edge phase (per layer) ----------------
            def emit_layer(l, ta, tb, cc_mid=None, cc_end=None):
                lo_ap = ta.ap()
                hi_ap = tb.ap()
                # last B-stream tile that finalizes a window < wsplit: the
                # point after which the first-half AllGather can fire
                ccmid_tile = -1
                for w in range(min(wsplit, nwin)):
                    if (1, w) in burst:
                        ccmid_tile = max(ccmid_tile, burst[(1, w)][1])
                if ccmid_tile < 0:
                    ccmid_tile = int(sec_len[0].sum()) // 128 - 1
                awb = cp.tile([128, nwin, 128], BF16, tag=f"awb{l}")
                with tc.tile_pool(name=f"g{l}", bufs=int(os.environ.get("KGB", "4"))) as gp, \
                     tc.tile_pool(name=f"scr{l}", bufs=6) as scrp, \
                     tc.tile_pool(name=f"oh{l}", bufs=3) as ohp, \
                     tc.tile_pool(name=f"fin{l}", bufs=2) as fp, \
                     tc.tile_pool(name=f"aw{l}", bufs=4) as awp, \
                     tc.tile_pool(name=f"awps{l}", bufs=2, space="PSUM") as awps, \
                     tc.tile_pool(name=f"psW{l}", bufs=2, space="PSUM") as psW:
                    # per-window broadcast of the a-term: awb[p, w, f] = a_arr[f, w, l]
                    for w in range(nwin):
                        tp_ps = awps.tile([1, 128], F32, tag="aT")
                        nc.tensor.matmul(tp_ps[:], a_arr[:, w, l:l + 1], identb_sb[:],
                                         start=True, stop=True)
                        aT = awp.tile([1, 128], F32, tag="aTs")
                        nc.vector.tensor_copy(aT[:], tp_ps[:])
                        ab_ps = awps.tile([128, 128], F32, tag="ab")
                        nc.tensor.matmul(ab_ps[:], ones_sb[:], aT[:],
                                         start=True, stop=True)
                        nc.scalar.activation(awb[:, w, :], ab_ps[:],
                                             mybir.ActivationFunctionType.Copy)

                    def finalize(w):
                        rows = 128 if w < nwin - 1 else last_win_rows
                        if l == 0:
                            ext1 = fp.tile([128, EXT_SLOTS], F32, tag="ext1")
                            nc.scalar.activation(ext1[:, 0:hq].bitcast(FP8), acc[:, w, :],
                                                 mybir.ActivationFunctionType.Copy,
                                                 scale=dinvc_sb[:, w:w + 1])
                            scr = scrp.tile([128, hidden], BF16, tag="escr")
                            b_f = scrp.tile([128, 1], F32, tag="bf1")
                            nc.vector.scalar_tensor_tensor(
                                out=scr[:], in0=acc[:, w, :], scalar=1.0, in1=gw_sb[:, 3, :],
                                op0=mybir.AluOpType.mult, op1=mybir.AluOpType.mult,
                                accum_out=b_f[:])
                            nc.scalar.activation(ext1[:, B_SLOT:].bitcast(BF16),
                                                 ones128b_sb[:, 0:64],
                                                 mybir.ActivationFunctionType.Copy,
                                                 scale=b_f[:])
                            scr2 = scrp.tile([128, hidden], BF16, tag="escr2")
                            a_f = scrp.tile([128, 1], F32, tag="af1")
                            nc.vector.scalar_tensor_tensor(
                                out=scr2[:], in0=acc[:, w, :], scalar=1.0, in1=gw_sb[:, 2, :],
                                op0=mybir.AluOpType.mult, op1=mybir.AluOpType.mult,
                                accum_out=a_f[:])
                            nc.vector.tensor_scalar(out=a_arr[:, w, 1:2], in0=a_f[:],
                                                    scalar1=gb_sb[:, 1:2], scalar2=None,
                                                    op0=mybir.AluOpType.add)
                            nc.sync.dma_start(agi.ap()[w * 128:w * 128 + rows, :],
                                              ext1[0:rows, :])

                    # windows with no lo-burst: seed acc with rawsc; fully
                    # edgeless windows also finalize immediately
                    for w in range(nwin):
                        if (0, w) not in burst:
                            nc.vector.tensor_copy(acc[:, w, :], rawsc[:, w, :])
                            if (1, w) not in burst:
                                finalize(w)

                    qi = 0
                    W_ps = None
                    for (h, ts, nt) in [c for c in calls]:
                        ct = nt // 128
                        t0 = ts // 128
                        G = gp.tile([128, CT_MAX, EXT_SLOTS], F32, tag="G")
                        nc.gpsimd.dma_gather(
                            out_ap=G[:, 0:ct, :],
                            in_ap=(hi_ap if h else lo_ap),
                            idxs_ap=idxt[:, ts // 16:(ts + nt) // 16],
                            num_idxs=nt, num_idxs_reg=nt, elem_size=EXT_SLOTS,
                            single_packet=False, queue_num=qi % 4)
                        qi += 1
                        # split call into window-pure runs
                        runs = []
                        c = 0
                        while c < ct:
                            w = tiles_w[t0 + c]
                            c1 = c
                            while c1 < ct and tiles_w[t0 + c1] == w:
                                c1 += 1
                            runs.append((w, c, c1))
                            c = c1
                        # mask: eqc[p, c, f] = (iota[f]==rr[p,c]); rr=-1 pads
                        eqc = ohp.tile([128, CT_MAX, 128], BF16, tag="eqc")
                        rrs = rrb_sb[:, t0:t0 + ct]
                        rrx = bass.AP(rrs.tensor, rrs.offset, list(rrs.ap) + [[0, 128]])
                        nc.vector.tensor_tensor(out=eqc[:, 0:ct, :],
                                                in0=iotac_sb[:, 0:ct, :], in1=rrx,
                                                op=mybir.AluOpType.is_equal)
                        # dest-expanded gate arg: s1 = awb[p,f] + b[p,c]
                        s1 = ohp.tile([128, CT_MAX, 128], BF16, tag="s1")
                        for (w, c0, c1) in runs:
                            rK = c1 - c0
                            bx = G[:, c0:c1, B_SLOT:].bitcast(BF16)
                            for half in (0, 1):
                                aws = awb[:, w, 64 * half:64 * half + 64]
                                awx = bass.AP(aws.tensor, aws.offset,
                                              [list(aws.ap)[0], [0, rK],
                                               list(aws.ap)[1]])
                                nc.vector.tensor_tensor(
                                    out=s1[:, c0:c1, 64 * half:64 * half + 64],
                                    in0=bx, in1=awx,
                                    op=mybir.AluOpType.add)
                        th = ohp.tile([128, CT_MAX, 128], BF16, tag="th")
                        nc.scalar.activation(th[:, 0:ct, :], s1[:, 0:ct, :],
                                             mybir.ActivationFunctionType.Tanh)
                        oh1 = ohp.tile([128, CT_MAX, 128], BF16, tag="oh1")
                        nc.vector.tensor_tensor(out=oh1[:, 0:ct, :],
                                                in0=eqc[:, 0:ct, :], in1=th[:, 0:ct, :],
                                                op=mybir.AluOpType.mult)
                        ohc = ohp.tile([128, CT_MAX, 128], FP8, tag="ohc")
                        for (w, c0, c1) in runs:
                            rK = c1 - c0
                            dws = dinvw_sb[:, w, :]
                            dwx = bass.AP(dws.tensor, dws.offset,
                                          [list(dws.ap)[0], [0, rK], list(dws.ap)[1]])
                            nc.vector.tensor_tensor(out=ohc[:, c0:c1, :],
                                                    in0=oh1[:, c0:c1, :], in1=dwx,
                                                    op=mybir.AluOpType.mult)
                        # pass 2: one-hot scatter matmuls
                        for (w, c0, c1) in runs:
                            bf, bl = burst[(h, w)]
                            if t0 + c0 == bf:
                                W_ps = psW.tile([128, hidden], F32, tag="W")
                            for c in range(c0, c1):
                                nc.tensor.matmul(W_ps[:], ohc[:, c, :], G[:, c, 0:hq].bitcast(FP8),
                                                 start=(t0 + c == bf),
                                                 stop=(t0 + c == bl))
                            if t0 + c1 - 1 == bl:
                                if h == 0:
                                    nc.vector.tensor_tensor(out=acc[:, w, :], in0=W_ps[:],
                                                            in1=rawsc[:, w, :],
                                                            op=mybir.AluOpType.add)
                                    if last_stream[w] == 0:
                                        finalize(w)
                                else:
                                    nc.vector.tensor_tensor(out=acc[:, w, :], in0=W_ps[:],
                                                            in1=acc[:, w, :],
                                                            op=mybir.AluOpType.add)
                                    finalize(w)
                        if cc_mid is not None and t0 <= ccmid_tile < t0 + ct:
                            cc_mid()
                    if cc_end is not None:
                        cc_end()

            if plvl >= 1:
                emit_layer(
                    0, ext0a, ext0b,
                    cc_mid=(lambda: cc_ag(agi.ap()[0:arows, :], agoa))
                    if plvl >= 2 else None,
                    cc_end=(lambda: cc_ag(agi.ap()[arows:r_per, :], agob))
                    if plvl >= 2 else None)
            if plvl >= 3:
                emit_layer(1, agoa, agob)
            if plvl < 4:
                with tc.tile_pool(name="zout", bufs=1) as zp:
                    o_z = zp.tile([128, out_ch], F32, tag="oz")
                    nc.vector.memset(o_z[:], 0.0)
                    for w in range(nwin):
                        rows = 128 if w < nwin - 1 else last_win_rows
                        nc.sync.dma_start(out.ap()[w * 128:w * 128 + rows, :],
                                          o_z[0:rows, :])
                return nc

            # ---------------- head: out = log_softmax(h @ t2^T + b) ----------
            # two passes so the Act engine loads the Exp/Ln tables once each
            with tc.tile_pool(name="head", bufs=4) as hp, \
                 tc.tile_pool(name="head_ps", bufs=4, space="PSUM") as hps:
                o_all = cp.tile([128, nwin, out_ch], F32, tag="o_all")
                nm_all = cp.tile([128, nwin], F32, tag="nm_all")
                s_all = cp.tile([128, nwin], F32, tag="s_all")
                for w in range(nwin):
                    ht_ps = hps.tile([128, 128], F32, tag="ht")
                    nc.tensor.matmul(ht_ps[:], acc[:, w, :], ident_sb[:],
                                     start=True, stop=True)
                    ht_sb = hp.tile([128, 128], F32, tag="ht_sb")
                    nc.vector.tensor_copy(ht_sb[:], ht_ps[:])
                    o_ps = hps.tile([128, out_ch], F32, tag="ops")
                    nc.tensor.matmul(o_ps[:], ht_sb[:], t2wt_sb[:], start=True, stop=False)
                    nc.tensor.matmul(o_ps[:], ones_sb[:], t2b_sb[:], start=False, stop=True)
                    nc.vector.reduce_max(out=nm_all[:, w:w + 1], in_=o_ps[:],
                                         axis=mybir.AxisListType.X, negate=True)
                    e_sb = hp.tile([128, out_ch], F32, tag="e")
                    nc.scalar.activation(e_sb[:], o_ps[:],
                                         mybir.ActivationFunctionType.Exp,
                                         bias=nm_all[:, w:w + 1])
                    nc.vector.reduce_sum(out=s_all[:, w:w + 1], in_=e_sb[:],
                                         axis=mybir.AxisListType.X)
                    nc.vector.tensor_copy(o_all[:, w, :], o_ps[:])
                ls_all = cp.tile([128, nwin], F32, tag="ls_all")
                nc.scalar.activation(ls_all[:], s_all[:], mybir.ActivationFunctionType.Ln)
                for w in range(nwin):
                    rows = 128 if w < nwin - 1 else last_win_rows
                    o_sb = hp.tile([128, out_ch], F32, tag="o")
                    nc.vector.tensor_scalar(out=o_sb[:], in0=o_all[:, w, :],
                                            scalar1=nm_all[:, w:w + 1],
                                            scalar2=ls_all[:, w:w + 1],
                                            op0=mybir.AluOpType.add,
                                            op1=mybir.AluOpType.subtract)
                    nc.sync.dma_start(out.ap()[w * 128:w * 128 + rows, :], o_sb[0:rows, :])

    return nc


# ======================================================================
# Host driver
# ======================================================================

def _bf16(a):
    import ml_dtypes
    return np.asarray(a, dtype=ml_dtypes.bfloat16)


def _group_x(xT_pad, nrow_units, kt):
    # xT_pad: [in_ch+1, units*128] f32 -> xtg [ngrp, 128, PREP_GRP*kt, 128],
    # xbg [ngrp, 1, PREP_GRP, 128] (ones row)
    in_ch = (xT_pad.shape[0] - 1)
    ngrp_t = (nrow_units + PREP_GRP - 1) // PREP_GRP
    pad_units = ngrp_t * PREP_GRP
    xp = np.zeros((in_ch + 1, pad_units * 128), np.float32)
    xp[:, :xT_pad.shape[1]] = xT_pad
    # [in, u, 128] -> [u, in, 128]
    xr = xp[:in_ch].reshape(in_ch, pad_units, 128).transpose(1, 0, 2)
    # [g, c, k, p, r] with in = k*128+p
    xg = xr.reshape(ngrp_t, PREP_GRP, kt, 128, 128)
    xtg = np.ascontiguousarray(xg.transpose(0, 3, 1, 2, 4)).reshape(
        ngrp_t, 128, PREP_GRP * kt, 128)
    xb = xp[in_ch].reshape(ngrp_t, 1, PREP_GRP, 128)
    return _bf16(xtg), _bf16(np.ascontiguousarray(xb))


def kernel_run(x, edge_index, t1_w, t1_b, gate_w, gate_b, t2_w, t2_b,
               n_nodes=N_NODES, in_ch=IN_CH, hidden=HIDDEN, out_ch=OUT_CH,
               eps=EPS, ncores=NCORES, lo_split=None, trace=False):
    _install_profile_hook()
    from concourse import bass_utils

    if lo_split is None:
        lo_split = min(25000, ((n_nodes + 1) // 2 + 127) // 128 * 128)
    meta = preprocess(edge_index, n_nodes, ncores, lo_split)
    nwin = meta["nwin"]
    r_per = n_nodes // ncores
    kt = in_ch // 128
    dinv = meta["dinv"]

    nc = build_kernel(meta, n_nodes, in_ch, hidden, out_ch, eps, lo_split, ncores)
    nc.finalize()

    # host arrays
    x = np.asarray(x, np.float32)
    xT = np.concatenate([x.T, np.ones((1, x.shape[0]), np.float32)], axis=0)  # [in+1, N]

    t1wt_h = _bf16(np.concatenate([np.asarray(t1_w, np.float32).T,
                                   np.asarray(t1_b, np.float32)[None, :]], axis=0))
    gw = np.asarray(gate_w, np.float32)
    gwrep_h = _bf16(np.stack([
        np.tile(gw[0, :hidden][None, :], (128, 1)),
        np.tile(gw[0, hidden:][None, :], (128, 1)),
        np.tile(gw[1, :hidden][None, :], (128, 1)),
        np.tile(gw[1, hidden:][None, :], (128, 1))]))
    gbrep_h = np.tile(np.asarray(gate_b, np.float32)[None, :], (128, 1))
    t2wt_h = np.ascontiguousarray(np.asarray(t2_w, np.float32).T)
    t2b_h = np.asarray(t2_b, np.float32)[None, :]
    iotac_h = _bf16(np.tile(np.arange(128, dtype=np.float32)[None, :],
                            (128, CT_MAX)))
    ident_h = np.eye(128, dtype=np.float32)
    identb_h = _bf16(np.eye(128, dtype=np.float32))
    ones_h = np.ones((1, 128), np.float32)
    ones128b_h = _bf16(np.ones((128, 128), np.float32))

    dinv_pad = np.zeros(nwin * 128 * ncores + 128, np.float32)
    dinv_pad[:n_nodes] = dinv[:n_nodes] if dinv.shape[0] >= n_nodes else 0

    in_maps = []
    for c in range(ncores):
        sl = np.zeros((in_ch + 1, nwin * 128), np.float32)
        take = min(nwin * 128, xT.shape[1] - c * r_per)
        sl[:, :take] = xT[:, c * r_per: c * r_per + take]
        xtog_h, xbog_h = _group_x(sl, nwin, kt)
        # own-row dinv, padded to whole windows with zeros
        dv = np.zeros(nwin * 128, np.float32)
        dv[:min(r_per, nwin * 128)] = dinv[c * r_per:c * r_per + r_per]
        dinvw_h = _bf16(np.tile(dv[None, :], (128, 1)))
        dinvc_h = np.ascontiguousarray(dv.reshape(nwin, 128).T)  # [128, nwin]
        in_maps.append({
            "xtog": xtog_h, "xbog": xbog_h,
            "t1wt": t1wt_h, "gwrep": gwrep_h, "gbrep": gbrep_h,
            "t2wt": t2wt_h, "t2b": t2b_h,
            "iotac": iotac_h, "ident": ident_h, "identb": identb_h,
            "ones": ones_h, "ones128b": ones128b_h,
            "idx": meta["idx_dev"][c],
            "rrb": _bf16(meta["rr_dev"][c]),
            "dinvw": dinvw_h, "dinvc": dinvc_h,
        })

    res = bass_utils.run_bass_kernel_spmd(
        nc, in_maps, core_ids=list(range(ncores)), trace=trace)
    outp = np.concatenate([res.results[c]["out"] for c in range(ncores)], axis=0)
    return outp[:n_nodes], res


def kernel(**inputs):
    x = inputs["x"]
    edge_index = inputs["edge_index"]
    outp, _ = kernel_run(
        x, edge_index, inputs["t1_w"], inputs["t1_b"], inputs["gate_w"],
        inputs["gate_b"], inputs["t2_w"], inputs["t2_b"])
    return np.asarray(outp, np.float32)
